# revision 1
# baseline (speedup 1.0000x reference)
"""Dilated segment attention on 8 Trainium2 NeuronCores (Bass/Tile).

Problem: x:[4,8192,1024] fp32. Per 64-token segment, rows ::2 are kept
(32 tokens), projected with Wq/Wk/Wv (+bias), and full-dim attention is
computed within each segment. Output: [4,4096,1024] fp32.

Sharding: data-parallel. Core c handles batch c//2, sequence half c%2 ->
2048 dilated tokens = 64 segments. No collectives. The host passes the
weights transposed ([d_in, d_out] layout, a pure layout prep like the
bias reshape) and per-core contiguous x slices.

Per-core pipeline (all matmuls bf16, fp32 PSUM accumulation):
  - SWDGE cast-DMAs (fp32->bf16): W.T row-tiles straight into SBUF;
    the dilated x rows into DRAM scratch, then big xbar DMA-transposes
    (DRAM->SBUF, one per (512-token chunk, 128-d stripe)) build
    x.T [d_in, tok] in SBUF. A short junk-matmul warm-up keeps the PE
    HAM at 2.4 GHz while the first transfers land.
  - Weight-stationary passes -> q.T, k.T [d_out, tok] (bias fused into
    the ACT psum->sbuf epilogue), chunk-outer so compute starts on chunk
    0 while later chunks stream. x-stationary pass -> v [tok, d_out].
  - simT per 4-segment group as one packed 128x128 matmul over 8 k-tiles
    (diagonal 32x32 blocks are the real per-segment logits; logits are
    bounded ~[-1.6,1.6] so no max-subtraction is needed). ACT computes
    p = exp(scale*simT) from PSUM into a zeroed tile, diagonal blocks
    only, so full-width K=128 matmuls against p contract the off-diag
    zeros away.
  - attn@v and the softmax denominator l (ones-column matmul) per token
    tile; final out = psum_av * (1/l) + bv in one DVE
    scalar_tensor_tensor (v is projected without bias: softmax rows sum
    to 1, so + bv after).
"""

import numpy as np

P = 128
D = 1024
KT = 8  # d_in tiles of 128
OT = 8  # d_out tiles of 128
NTT = 16  # token tiles of 128 (2048 tokens per core)
FD = 512  # matmul moving free dim / psum bank
TCH = 4  # token chunks of 512

_CACHE = {}


def _build_nc():
    import os
    from contextlib import ExitStack

    import concourse.bass as bass
    import concourse.mybir as mybir
    import concourse.tile as tile
    from concourse import bacc

    KPHASE = int(os.environ.get("KPHASE", "5"))
    NWARM = int(os.environ.get("KWARM", "0"))

    dt = mybir.dt
    AF = mybir.ActivationFunctionType
    ALU = mybir.AluOpType

    nc = bacc.Bacc("TRN2", target_bir_lowering=False, debug=False,
                   enable_asserts=False)

    x_d = nc.dram_tensor("x", [4096, D], dt.float32, kind="ExternalInput")
    wqt_d = nc.dram_tensor("wqt", [D, D], dt.float32, kind="ExternalInput")
    wkt_d = nc.dram_tensor("wkt", [D, D], dt.float32, kind="ExternalInput")
    wvt_d = nc.dram_tensor("wvt", [D, D], dt.float32, kind="ExternalInput")
    bq_d = nc.dram_tensor("bqr", [P, OT], dt.float32, kind="ExternalInput")
    bk_d = nc.dram_tensor("bkr", [P, OT], dt.float32, kind="ExternalInput")
    bv_d = nc.dram_tensor("bvb", [1, D], dt.bfloat16, kind="ExternalInput")
    out_d = nc.dram_tensor("out", [2048, D], dt.float32, kind="ExternalOutput")

    wt_dram = [wqt_d, wkt_d, wvt_d]
    scale = float(D) ** -0.5

    with tile.TileContext(nc) as tc, ExitStack() as ctx:
        consts = ctx.enter_context(tc.tile_pool(name="consts", bufs=1))
        resid = ctx.enter_context(tc.tile_pool(name="resid", bufs=1))
        wtp = ctx.enter_context(tc.tile_pool(name="wtp", bufs=2))
        wstage = ctx.enter_context(tc.tile_pool(name="wstage", bufs=4))
        outp = ctx.enter_context(tc.tile_pool(name="outp", bufs=4))
        rsbp = ctx.enter_context(tc.tile_pool(name="rsbp", bufs=2))
        dpool = ctx.enter_context(tc.tile_pool(name="dram", bufs=1,
                                               space="DRAM"))

        ones_col = consts.tile([P, 1], dt.bfloat16, name="ones_col")
        ones_row = consts.tile([1, P], dt.bfloat16, name="ones_row")
        bq_sb = consts.tile([P, OT], dt.float32, name="bq_sb")
        bk_sb = consts.tile([P, OT], dt.float32, name="bk_sb")
        bvb_sb = consts.tile([1, D], dt.bfloat16, name="bvb_sb")
        bv_rep = consts.tile([P, D], dt.float32, name="bv_rep")

        nc.vector.memset(ones_col[:], 1.0)
        nc.vector.memset(ones_row[:], 1.0)
        nc.sync.dma_start(bq_sb[:], bq_d[:])
        nc.sync.dma_start(bk_sb[:], bk_d[:])
        nc.sync.dma_start(bvb_sb[:], bv_d[:])

        xT = [resid.tile([P, 2048], dt.bfloat16, name=f"xT{k}") for k in range(KT)]
        qT = [resid.tile([P, 2048], dt.bfloat16, name=f"qT{o}") for o in range(OT)]
        kT = [resid.tile([P, 2048], dt.bfloat16, name=f"kT{o}") for o in range(OT)]
        vv = [resid.tile([P, D], dt.bfloat16, name=f"v{t}") for t in range(NTT)]
        pT = [resid.tile([P, P], dt.bfloat16, name=f"pT{g}") for g in range(NTT)]

        # pT holds block-diagonal exp(sim) — zero once, exp writes only the
        # diagonal 32x32 blocks, so full-width (K=128) attn@v and l matmuls
        # contract the zeros away.
        for g in range(NTT):
            nc.vector.memset(pT[g][:], 0.0)

        # ---- x: dilated rows cast to bf16 DRAM scratch (SWDGE), then big
        # xbar transposes DRAM->SBUF build x.T. All transposes stay on the
        # sync HWDGE queue (concurrent xbar transposes from two queues
        # corrupt data — measured).
        x_bf = dpool.tile([2048, D], dt.bfloat16, name="x_bf")

        def cast_x_chunk(c):
            # 512 dilated tokens: 4 token tiles x (4 segs x 32 rows of ::2)
            src = bass.AP(x_d, 1024 * D * c,
                          [[256 * D, 4], [64 * D, 4], [2 * D, 32], [1, D]])
            return nc.gpsimd.dma_start(x_bf[512 * c:512 * c + 512, :], src)

        def load_wT(j, swdge=False):
            """W.T [d_in, d_out] bf16 row-tiles from the host-transposed
            weights. swdge=True: direct cast-DMA (half the bytes — used for
            Wq whose latency gates the start; the descriptor ring has room
            early). Otherwise: plain fp32 loads on the sync HWDGE queue
            (keeps the SWDGE ring free for the x casts) + ACT cast to bf16.
            Pool tags shared across passes reuse the same slots."""
            wts = []
            for i in range(KT):
                src = bass.AP(wt_dram[j], i * P * D, [[D, P], [1, D]])
                wt = wtp.tile([P, D], dt.bfloat16, name=f"wT{i}")
                if swdge:
                    nc.gpsimd.dma_start(wt[:], src)
                else:
                    stg = wstage.tile([P, D], dt.float32, name="wstg")
                    nc.sync.dma_start(stg[:], src)
                    nc.vector.tensor_copy(wt[:], stg[:])
                wts.append(wt)
            return wts

        def transpose_x_chunk(c):
            for k in range(KT):
                nc.sync.dma_start(xT[k][:, FD * c:FD * c + FD],
                                  x_bf[FD * c:FD * c + FD, P * k:P * k + P],
                                  transpose=True)

        # SWDGE order = demand order: x chunk 0, Wq.T tiles, then the rest.
        # xc2/xc3 are held behind xc0/xc1 completion so the critical first
        # chunk + Wq loads get the SDMA bandwidth (concurrent DMAs share it
        # round-robin).
        from concourse.bass import _add_dep_helper

        xc0 = cast_x_chunk(0)
        wq_ts = load_wT(0)
        xc1 = cast_x_chunk(1)
        xc2 = cast_x_chunk(2)
        _add_dep_helper(xc2.ins, xc0.ins, reason="throttle xc2 behind xc0")
        xc3 = cast_x_chunk(3)
        _add_dep_helper(xc3.ins, xc1.ins, reason="throttle xc3 behind xc1")
        transpose_x_chunk(0)

        if KPHASE >= 2:
            if NWARM:
                # HAM warm-up: full-K junk matmuls so real matmuls start at
                # 2.4 GHz. (K=1 fillers don't register as PE activity.)
                junk_w = consts.tile([P, P], dt.bfloat16, name="junk_w")
                junk_m = consts.tile([P, FD], dt.bfloat16, name="junk_m")
                nc.vector.memset(junk_w[:], 0.0)
                nc.vector.memset(junk_m[:], 0.0)
                with tc.tile_pool(name="warm", bufs=1, space="PSUM") as wp:
                    wps = wp.tile([P, FD], dt.float32, name="wps")
                    for _ in range(NWARM):
                        nc.tensor.matmul(wps[:], junk_w[:], junk_m[:],
                                         start=True, stop=True)

            with tc.tile_pool(name="ppool", bufs=6, space="PSUM") as ppool, \
                 tc.tile_pool(name="spool", bufs=2, space="PSUM") as spool:

                # ---- bv broadcast to all partitions via K=1 ones matmul
                for dh in range(2):
                    ps = ppool.tile([P, FD], dt.float32, name="pps")
                    nc.tensor.matmul(ps[:], ones_row[:],
                                     bvb_sb[:, FD * dh:FD * dh + FD],
                                     start=True, stop=True)
                    nc.scalar.copy(bv_rep[:, FD * dh:FD * dh + FD], ps[:])

                # ---- q/k passes: weights stationary, x.T moving -> q.T/k.T
                # chunk-outer so the PE starts on chunk 0 while later x
                # chunks are still being cast/transposed.
                def proj_pass(j, b_sb, dstT, wts=None):
                    if wts is None:
                        wts = load_wT(j)
                    if j == 0:
                        for c in range(1, TCH):
                            transpose_x_chunk(c)
                    for c in range(TCH):
                        for o in range(OT):
                            pss = ppool.tile([P, FD], dt.float32, name="pps")
                            for i in range(KT):
                                nc.tensor.matmul(pss[:],
                                                 wts[i][:, P * o:P * o + P],
                                                 xT[i][:, FD * c:FD * c + FD],
                                                 start=(i == 0),
                                                 stop=(i == KT - 1))
                            nc.scalar.activation(dstT[o][:, FD * c:FD * c + FD],
                                                 pss[:], AF.Identity,
                                                 bias=b_sb[:, o:o + 1],
                                                 scale=1.0)

                proj_pass(0, bq_sb, qT, wts=wq_ts)
                if KPHASE >= 3:
                    proj_pass(1, bk_sb, kT)

                    # ---- simT per 4-seg group; p = exp(scale*simT) via ACT
                    # (only the diagonal blocks — pT stays 0 elsewhere)
                    for g in range(NTT):
                        sps = spool.tile([P, P], dt.float32, name="sps")
                        for kk in range(KT):
                            nc.tensor.matmul(sps[:], kT[kk][:, P * g:P * g + P],
                                             qT[kk][:, P * g:P * g + P],
                                             start=(kk == 0),
                                             stop=(kk == KT - 1))
                        for a in range(4):
                            nc.scalar.activation(
                                pT[g][32 * a:32 * a + 32, 32 * a:32 * a + 32],
                                sps[32 * a:32 * a + 32, 32 * a:32 * a + 32],
                                AF.Exp, bias=0.0, scale=scale)

        if KPHASE >= 4:
            # ---- v pass interleaved with attn@v: AV group t only needs
            # vv[t] (just produced) and pT[t] (from the sim phase), so each
            # AV group hides behind the next v tile's matmuls instead of
            # running serially at the end. Block-diag pT makes the
            # full-width K=128 AV and l matmuls exact.
            with tc.tile_pool(name="vpool", bufs=3, space="PSUM") as vpool, \
                 tc.tile_pool(name="avp", bufs=4, space="PSUM") as avp, \
                 tc.tile_pool(name="lp", bufs=1, space="PSUM") as lp:
                wvs = load_wT(2)
                for t in range(NTT):
                    pss = [vpool.tile([P, FD], dt.float32, name="pps")
                           for _ in range(2)]
                    for i in range(KT):
                        for dh in range(2):
                            nc.tensor.matmul(pss[dh][:],
                                             xT[i][:, P * t:P * t + P],
                                             wvs[i][:, FD * dh:FD * dh + FD],
                                             start=(i == 0),
                                             stop=(i == KT - 1))
                    for dh in range(2):
                        nc.vector.tensor_copy(
                            vv[t][:, FD * dh:FD * dh + FD], pss[dh][:])
                    if KPHASE >= 5:
                        lps = lp.tile([P, 1], dt.float32, name="lps")
                        nc.tensor.matmul(lps[:], pT[t][:], ones_col[:],
                                         start=True, stop=True)
                        rsb = rsbp.tile([P, 1], dt.float32, name="rsb")
                        nc.vector.reciprocal(rsb[:], lps[:])
                        osb = outp.tile([P, D], dt.float32, name="osb")
                        for dh in range(2):
                            avs = avp.tile([P, FD], dt.float32, name="avs")
                            nc.tensor.matmul(avs[:], pT[t][:],
                                             vv[t][:, FD * dh:FD * dh + FD],
                                             start=True, stop=True)
                            nc.vector.scalar_tensor_tensor(
                                osb[:, FD * dh:FD * dh + FD], avs[:], rsb[:],
                                bv_rep[:, FD * dh:FD * dh + FD],
                                ALU.mult, ALU.add)
                        nc.sync.dma_start(
                            bass.AP(out_d, t * P * D, [[D, P], [1, D]]),
                            osb[:])
        if KPHASE < 5:
            dmp = outp.tile([P, D], dt.float32, name="osb")
            nc.vector.memset(dmp[:], 0.0)
            nc.sync.dma_start(bass.AP(out_d, 0, [[D, P], [1, D]]), dmp[:])

    nc.compile()
    return nc


def get_nc():
    if "nc" not in _CACHE:
        _CACHE["nc"] = _build_nc()
    return _CACHE["nc"]


def make_in_maps(x, Wq, bq, Wk, bk, Wv, bv):
    import ml_dtypes

    x = np.asarray(x, np.float32)
    wqt = np.ascontiguousarray(np.asarray(Wq, np.float32).T)
    wkt = np.ascontiguousarray(np.asarray(Wk, np.float32).T)
    wvt = np.ascontiguousarray(np.asarray(Wv, np.float32).T)
    bqr = np.ascontiguousarray(np.asarray(bq, np.float32).reshape(OT, P).T)
    bkr = np.ascontiguousarray(np.asarray(bk, np.float32).reshape(OT, P).T)
    bvb = np.asarray(bv, np.float32).reshape(1, D).astype(ml_dtypes.bfloat16)
    in_maps = []
    for c in range(8):
        b, h = divmod(c, 2)
        xs = np.ascontiguousarray(x[b, 4096 * h:4096 * h + 4096, :])
        in_maps.append({"x": xs, "wqt": wqt, "wkt": wkt, "wvt": wvt,
                        "bqr": bqr, "bkr": bkr, "bvb": bvb})
    return in_maps


def kernel(x, Wq, bq, Wk, bk, Wv, bv):
    from concourse.bass_utils import run_bass_kernel_spmd

    nc = get_nc()
    in_maps = make_in_maps(x, Wq, bq, Wk, bk, Wv, bv)
    res = run_bass_kernel_spmd(nc, in_maps, core_ids=list(range(8)))
    _CACHE["last_res"] = res
    out = np.empty((4, 4096, D), np.float32)
    for c in range(8):
        b, h = divmod(c, 2)
        out[b, 2048 * h:2048 * h + 2048] = res.results[c]["out"]
    return out



# revision 6
# speedup vs baseline: 1.2065x; 1.2065x over previous
"""Dilated segment attention on 8 Trainium2 NeuronCores (Bass/Tile).

Problem: x:[4,8192,1024] fp32. Per 64-token segment, rows ::2 are kept
(32 tokens), projected with Wq/Wk/Wv (+bias), and full-dim attention is
computed within each segment. Output: [4,4096,1024] fp32.

Sharding: data-parallel. Core c handles batch c//2, sequence half c%2 ->
2048 dilated tokens = 64 segments. No collectives.

Key algebraic restructure: softmax over keys cancels every term of
q_j.k_i that is constant in the key index i, so

  softmax_i(q_j . k_i) = softmax_i( x_j A x_i^T + x_i . w ),
  A = Wq^T Wk,  w = bq Wk        (bk drops out entirely).

The kernel computes A on-device once per core (1024^3 MACs, hidden in
the DMA-bound startup window) and replaces BOTH the q and k projections
with a single h-projection h = x A + w; simT[i,j] = x_i . h_j. This
cuts projection matmul work from 3 passes to 2 (plus the cheap A).

Host prep is layout/dtype only: weights passed bf16 (native Wq/Wk for
the A matmul, Wv^T for the v pass), x passed dilated+transposed+bf16 in
chunk-major [4*1024, 512] so every DMA is a contiguous 128KB tile load.

Per-core pipeline (all matmuls bf16, fp32 PSUM):
  - warm-up junk matmuls keep the PE HAM at 2.4 GHz while first DMAs land
  - v-pass for token tiles 0..3 (needs only Wv + x chunk 0)
  - w = bq Wk (64 tiny matmuls), A = Wq^T Wk (128 matmuls, N=512)
  - per chunk c: h-pass (A stationary, x.T moving) with w fused into the
    ACT psum->sbuf epilogue; simT per 4-segment group as one packed
    128x128 matmul over 8 k-tiles plus a rank-4 mask matmul that puts
    -30000 on the off-diagonal 32x32 blocks, so a single full-tile ACT
    exp yields the block-diagonal p (off-diag underflows to exactly 0);
    then v-pass for the next chunk's tiles and attn@v + the softmax
    denominator (ones-column matmul) per token tile; final
    out = psum_av * (1/l) + bv in one DVE scalar_tensor_tensor.
"""

import numpy as np

P = 128
D = 1024
KT = 8    # d tiles of 128
OT = 8    # d_out tiles of 128
NTT = 16  # token tiles of 128 (2048 tokens per core)
FD = 512  # matmul moving free dim / psum bank
TCH = 4   # token chunks of 512
NEG = -30000.0  # off-diagonal mask; exp(scale*(sim+NEG)) underflows to 0

_CACHE = {}


def _build_nc():
    import os
    from contextlib import ExitStack

    import concourse.bass as bass
    import concourse.mybir as mybir
    import concourse.tile as tile
    from concourse import bacc

    KWARM = int(os.environ.get("KWARM", "96"))

    dt = mybir.dt
    AF = mybir.ActivationFunctionType
    ALU = mybir.AluOpType

    nc = bacc.Bacc("TRN2", target_bir_lowering=False, debug=False,
                   enable_asserts=False)

    # x.T, dilated, bf16, chunk-major: row 1024*c + d holds x.T[d, 512c:...]
    xt_d = nc.dram_tensor("xt", [TCH * D, FD], dt.bfloat16,
                          kind="ExternalInput")
    wq_d = nc.dram_tensor("wqn", [D, D], dt.bfloat16, kind="ExternalInput")
    wk_d = nc.dram_tensor("wkn", [D, D], dt.bfloat16, kind="ExternalInput")
    wv_d = nc.dram_tensor("wvt", [D, D], dt.bfloat16, kind="ExternalInput")
    bqc_d = nc.dram_tensor("bqc", [P, KT], dt.bfloat16, kind="ExternalInput")
    bvb_d = nc.dram_tensor("bvb", [1, D], dt.bfloat16, kind="ExternalInput")
    mskl_d = nc.dram_tensor("mskl", [4, P], dt.bfloat16, kind="ExternalInput")
    mskr_d = nc.dram_tensor("mskr", [4, P], dt.bfloat16, kind="ExternalInput")
    out_d = nc.dram_tensor("out", [2048, D], dt.float32, kind="ExternalOutput")

    scale = float(D) ** -0.5

    with tile.TileContext(nc) as tc, ExitStack() as ctx:
        consts = ctx.enter_context(tc.tile_pool(name="consts", bufs=1))
        resid = ctx.enter_context(tc.tile_pool(name="resid", bufs=1))
        wqp = ctx.enter_context(tc.tile_pool(name="wqp", bufs=1))
        wkp = ctx.enter_context(tc.tile_pool(name="wkp", bufs=1))
        wvp = ctx.enter_context(tc.tile_pool(name="wvp", bufs=1))
        outp = ctx.enter_context(tc.tile_pool(name="outp", bufs=3))
        rsbp = ctx.enter_context(tc.tile_pool(name="rsbp", bufs=2))

        ones_col = consts.tile([P, 1], dt.bfloat16, name="ones_col")
        ones_row = consts.tile([1, P], dt.bfloat16, name="ones_row")
        maskL = consts.tile([4, P], dt.bfloat16, name="maskL")
        maskR = consts.tile([4, P], dt.bfloat16, name="maskR")
        junk_w = consts.tile([P, P], dt.bfloat16, name="junk_w")
        junk_m = consts.tile([P, P], dt.bfloat16, name="junk_m")
        bqc_sb = consts.tile([P, KT], dt.bfloat16, name="bqc_sb")
        bvb_sb = consts.tile([1, D], dt.bfloat16, name="bvb_sb")
        w_sb = consts.tile([P, OT], dt.float32, name="w_sb")
        bv_rep = consts.tile([P, D], dt.float32, name="bv_rep")

        nc.vector.memset(ones_col[:], 1.0)
        nc.vector.memset(ones_row[:], 1.0)
        nc.vector.memset(junk_w[:], 0.0)
        nc.vector.memset(junk_m[:], 0.0)

        # ---- DMA: tiny biases + masks first, then x chunk 0 + Wv (gate the
        # early v-pass) on the sync HWDGE ring; Wk + Wq (gate w/A) on the
        # scalar HWDGE ring; out-writes go on the SWDGE ring later.
        nc.sync.dma_start(bqc_sb[:], bqc_d[:])
        nc.sync.dma_start(bvb_sb[:], bvb_d[:])
        nc.sync.dma_start(maskL[:], mskl_d[:])
        nc.sync.dma_start(maskR[:], mskr_d[:])

        xT = [resid.tile([P, 2048], dt.bfloat16, name=f"xT{k}")
              for k in range(KT)]
        A_sb = [resid.tile([P, D], dt.bfloat16, name=f"A{m}")
                for m in range(KT)]
        hT = [resid.tile([P, 2048], dt.bfloat16, name=f"hT{o}")
              for o in range(OT)]
        vv = [resid.tile([P, D], dt.bfloat16, name=f"v{t}") for t in range(NTT)]
        pT = [resid.tile([P, P], dt.bfloat16, name=f"pT{g}") for g in range(NTT)]

        def load_x_chunk(c):
            for k in range(KT):
                src = bass.AP(xt_d, (D * c + P * k) * FD, [[FD, P], [1, FD]])
                nc.sync.dma_start(xT[k][:, FD * c:FD * c + FD], src)

        def load_w(dst_pool, src_d, tag, eng):
            tiles = []
            for i in range(KT):
                src = bass.AP(src_d, i * P * D, [[D, P], [1, D]])
                wt = dst_pool.tile([P, D], dt.bfloat16, name=f"{tag}{i}")
                eng.dma_start(wt[:], src)
                tiles.append(wt)
            return tiles

        load_x_chunk(0)
        wv_sb = load_w(wvp, wv_d, "wv", nc.sync)
        wk_sb = load_w(wkp, wk_d, "wk", nc.scalar)
        wq_sb = load_w(wqp, wq_d, "wq", nc.scalar)
        for c in range(1, TCH):
            load_x_chunk(c)

        with tc.tile_pool(name="projp", bufs=3, space="PSUM") as projp, \
             tc.tile_pool(name="simp", bufs=2, space="PSUM") as simp, \
             tc.tile_pool(name="avp", bufs=2, space="PSUM") as avp, \
             tc.tile_pool(name="lp", bufs=1, space="PSUM") as lp:

            # ---- HAM warm-up: junk matmuls from t~0 so the PE clock is
            # at 2.4 GHz when the first real matmuls arrive.
            if KWARM:
                wps = projp.tile([P, FD], dt.float32, name="pps")
                for _ in range(KWARM):
                    nc.tensor.matmul(wps[:, 0:P], junk_w[:], junk_m[:],
                                     start=True, stop=True)

            # ---- bv broadcast to all partitions via K=1 ones matmul
            for dh in range(2):
                ps = projp.tile([P, FD], dt.float32, name="pps")
                nc.tensor.matmul(ps[:], ones_row[:],
                                 bvb_sb[:, FD * dh:FD * dh + FD],
                                 start=True, stop=True)
                nc.scalar.copy(bv_rep[:, FD * dh:FD * dh + FD], ps[:])

            def v_pass(t):
                pss = [projp.tile([P, FD], dt.float32, name="pps")
                       for _ in range(2)]
                for i in range(KT):
                    for dh in range(2):
                        nc.tensor.matmul(pss[dh][:],
                                         xT[i][:, P * t:P * t + P],
                                         wv_sb[i][:, FD * dh:FD * dh + FD],
                                         start=(i == 0), stop=(i == KT - 1))
                for dh in range(2):
                    nc.vector.tensor_copy(vv[t][:, FD * dh:FD * dh + FD],
                                          pss[dh][:])

            # early v tiles: need only x chunk 0 + Wv (first on sync ring)
            for t in range(4):
                v_pass(t)

            # ---- w = bq @ Wk, column m at a time (psum [P, OT])
            w_ps = lp.tile([P, OT], dt.float32, name="lps")
            for m in range(OT):
                for i in range(KT):
                    nc.tensor.matmul(w_ps[:, m:m + 1],
                                     wk_sb[i][:, P * m:P * m + P],
                                     bqc_sb[:, i:i + 1],
                                     start=(i == 0), stop=(i == KT - 1))
            nc.vector.tensor_copy(w_sb[:], w_ps[:])

            # ---- A = Wq^T @ Wk  (A[a, b] = sum_o Wq[o, a] Wk[o, b])
            for hf in range(2):
                for m in range(KT):
                    psA = projp.tile([P, FD], dt.float32, name="pps")
                    for i in range(KT):
                        nc.tensor.matmul(psA[:],
                                         wq_sb[i][:, P * m:P * m + P],
                                         wk_sb[i][:, FD * hf:FD * hf + FD],
                                         start=(i == 0), stop=(i == KT - 1))
                    nc.vector.tensor_copy(A_sb[m][:, FD * hf:FD * hf + FD],
                                          psA[:])

            def av_group(t):
                lps = lp.tile([P, 1], dt.float32, name="lps")
                nc.tensor.matmul(lps[:], pT[t][:], ones_col[:],
                                 start=True, stop=True)
                rsb = rsbp.tile([P, 1], dt.float32, name="rsb")
                nc.vector.reciprocal(rsb[:], lps[:])
                osb = outp.tile([P, D], dt.float32, name="osb")
                for dh in range(2):
                    avs = avp.tile([P, FD], dt.float32, name="avs")
                    nc.tensor.matmul(avs[:], pT[t][:],
                                     vv[t][:, FD * dh:FD * dh + FD],
                                     start=True, stop=True)
                    nc.vector.scalar_tensor_tensor(
                        osb[:, FD * dh:FD * dh + FD], avs[:], rsb[:],
                        bv_rep[:, FD * dh:FD * dh + FD],
                        ALU.mult, ALU.add)
                nc.gpsimd.dma_start(
                    bass.AP(out_d, t * P * D, [[D, P], [1, D]]), osb[:])

            for c in range(TCH):
                # h-pass for chunk c: h = x A + w, stored transposed
                for o in range(OT):
                    psH = projp.tile([P, FD], dt.float32, name="pps")
                    for i in range(KT):
                        nc.tensor.matmul(psH[:],
                                         A_sb[i][:, P * o:P * o + P],
                                         xT[i][:, FD * c:FD * c + FD],
                                         start=(i == 0), stop=(i == KT - 1))
                    nc.scalar.activation(hT[o][:, FD * c:FD * c + FD],
                                         psH[:], AF.Identity,
                                         bias=w_sb[:, o:o + 1], scale=1.0)
                # simT + exp for this chunk's 4 groups
                for g in range(4 * c, 4 * c + 4):
                    sps = simp.tile([P, P], dt.float32, name="sps")
                    nc.tensor.matmul(sps[:], maskL[:], maskR[:],
                                     start=True, stop=False)
                    for kk in range(KT):
                        nc.tensor.matmul(sps[:],
                                         xT[kk][:, P * g:P * g + P],
                                         hT[kk][:, P * g:P * g + P],
                                         start=False, stop=(kk == KT - 1))
                    nc.scalar.activation(pT[g][:], sps[:], AF.Exp,
                                         bias=0.0, scale=scale)
                # v for the next chunk's tiles, then attn@v for this chunk
                if c + 1 < TCH:
                    for t in range(4 * (c + 1), 4 * (c + 1) + 4):
                        v_pass(t)
                for t in range(4 * c, 4 * c + 4):
                    av_group(t)

    nc.compile()
    return nc


def get_nc():
    if "nc" not in _CACHE:
        _CACHE["nc"] = _build_nc()
    return _CACHE["nc"]


def make_in_maps(x, Wq, bq, Wk, bk, Wv, bv):
    import ml_dtypes

    bf16 = ml_dtypes.bfloat16
    x = np.asarray(x, np.float32)
    wqn = np.ascontiguousarray(np.asarray(Wq, np.float32).astype(bf16))
    wkn = np.ascontiguousarray(np.asarray(Wk, np.float32).astype(bf16))
    wvt = np.ascontiguousarray(np.asarray(Wv, np.float32).astype(bf16).T)
    bqc = np.ascontiguousarray(
        np.asarray(bq, np.float32).astype(bf16).reshape(KT, P).T)
    bvb = np.asarray(bv, np.float32).reshape(1, D).astype(bf16)
    # maskL[a, i] = 1 iff i in 32-block a; maskR[a, j] = NEG unless j in a
    blk = (np.arange(P) // 32)[None, :] == np.arange(4)[:, None]
    mskl = blk.astype(bf16)
    mskr = np.where(blk, 0.0, NEG).astype(bf16)
    in_maps = []
    for c in range(8):
        b, h = divmod(c, 2)
        xs = x[b, 4096 * h:4096 * h + 4096]
        xs = xs.reshape(64, 64, D)[:, ::2, :].reshape(2048, D).astype(bf16)
        # x.T in chunk-major rows: row 1024*c + d = x.T[d, 512c:512c+512]
        xt = np.ascontiguousarray(
            xs.T.reshape(D, TCH, FD).transpose(1, 0, 2)).reshape(TCH * D, FD)
        in_maps.append({"xt": xt, "wqn": wqn, "wkn": wkn, "wvt": wvt,
                        "bqc": bqc, "bvb": bvb, "mskl": mskl, "mskr": mskr})
    return in_maps


def kernel(x, Wq, bq, Wk, bk, Wv, bv):
    from concourse.bass_utils import run_bass_kernel_spmd

    nc = get_nc()
    in_maps = make_in_maps(x, Wq, bq, Wk, bk, Wv, bv)
    res = run_bass_kernel_spmd(nc, in_maps, core_ids=list(range(8)))
    _CACHE["last_res"] = res
    out = np.empty((4, 4096, D), np.float32)
    for c in range(8):
        b, h = divmod(c, 2)
        out[b, 2048 * h:2048 * h + 2048] = res.results[c]["out"]
    return out


# revision 7
# speedup vs baseline: 1.2264x; 1.0165x over previous
"""Dilated segment attention on 8 Trainium2 NeuronCores (Bass/Tile).

Problem: x:[4,8192,1024] fp32. Per 64-token segment, rows ::2 are kept
(32 tokens), projected with Wq/Wk/Wv (+bias), and full-dim attention is
computed within each segment. Output: [4,4096,1024] fp32.

Sharding: data-parallel. Core c handles batch c//2, sequence half c%2 ->
2048 dilated tokens = 64 segments. No collectives.

Key algebraic restructure: softmax over keys cancels every term of
q_j.k_i that is constant in the key index i, so

  softmax_i(q_j . k_i) = softmax_i( x_j A x_i^T + x_i . w ),
  A = Wq^T Wk,  w = bq Wk        (bk drops out entirely).

The kernel computes A on-device once per core (1024^3 MACs, started as
soon as the 4 MB of Wq/Wk bf16 lands) and replaces BOTH the q and k
projections with a single h-projection h = x A + w; simT[i,j] = x_i.h_j.
This cuts projection matmul work from 3 passes to 2 (plus the cheap A).

Host prep is layout/dtype only: weights passed bf16 (native Wq/Wk for
the A matmul, Wv^T for the v pass), x passed dilated+transposed+bf16 in
chunk-major [4*1024, 512] layout. All SBUF residents use single big
tiles so each load is ONE 1-2 MB DMA op (descriptor-efficient), halves
split across the two HWDGE rings (sync + scalar) in priority order
wk -> wq -> x chunk 0 / wv -> x chunks 1-3.

Per-core pipeline (all matmuls bf16, fp32 PSUM):
  - warm-up junk matmuls keep the PE HAM at 2.4 GHz until real work
  - w = bq Wk (64 tiny matmuls, needs only Wk), A = Wq^T Wk (128
    matmuls, N=512) -> A in SBUF bf16
  - per chunk c: v-pass (x stationary, Wv^T moving); h-pass (A
    stationary, x.T moving) with w fused into the ACT psum->sbuf
    epilogue; simT per 4-segment group as one packed 128x128 matmul
    over 8 k-tiles plus a rank-4 mask matmul that puts -30000 on the
    off-diagonal 32x32 blocks so one full-tile ACT exp yields the
    block-diagonal p (off-diag underflows to exactly 0); attn@v and the
    softmax denominator l (ones-column matmul) per token tile; final
    out = psum_av * (1/l) + bv in one DVE scalar_tensor_tensor, written
    out on the (by then idle) HWDGE rings.
"""

import numpy as np

P = 128
D = 1024
KT = 8    # d tiles of 128
OT = 8    # d_out tiles of 128
NTT = 16  # token tiles of 128 (2048 tokens per core)
FD = 512  # matmul moving free dim / psum bank
TCH = 4   # token chunks of 512
NEG = -30000.0  # off-diagonal mask; exp(scale*(sim+NEG)) underflows to 0

_CACHE = {}


def _build_nc():
    import os
    from contextlib import ExitStack

    import concourse.bass as bass
    import concourse.mybir as mybir
    import concourse.tile as tile
    from concourse import bacc

    KWARM = int(os.environ.get("KWARM", "128"))

    dt = mybir.dt
    AF = mybir.ActivationFunctionType
    ALU = mybir.AluOpType

    nc = bacc.Bacc("TRN2", target_bir_lowering=False, debug=False,
                   enable_asserts=False)

    # x.T, dilated, bf16, chunk-major: row 1024*c + d holds x.T[d, 512c:...]
    xt_d = nc.dram_tensor("xt", [TCH * D, FD], dt.bfloat16,
                          kind="ExternalInput")
    wq_d = nc.dram_tensor("wqn", [D, D], dt.bfloat16, kind="ExternalInput")
    wk_d = nc.dram_tensor("wkn", [D, D], dt.bfloat16, kind="ExternalInput")
    wv_d = nc.dram_tensor("wvt", [D, D], dt.bfloat16, kind="ExternalInput")
    bqc_d = nc.dram_tensor("bqc", [P, KT], dt.bfloat16, kind="ExternalInput")
    bvb_d = nc.dram_tensor("bvb", [1, D], dt.bfloat16, kind="ExternalInput")
    mskl_d = nc.dram_tensor("mskl", [4, P], dt.bfloat16, kind="ExternalInput")
    mskr_d = nc.dram_tensor("mskr", [4, P], dt.bfloat16, kind="ExternalInput")
    out_d = nc.dram_tensor("out", [2048, D], dt.float32, kind="ExternalOutput")

    scale = float(D) ** -0.5

    with tile.TileContext(nc) as tc, ExitStack() as ctx:
        consts = ctx.enter_context(tc.tile_pool(name="consts", bufs=1))
        resid = ctx.enter_context(tc.tile_pool(name="resid", bufs=1))
        outp = ctx.enter_context(tc.tile_pool(name="outp", bufs=3))
        rsbp = ctx.enter_context(tc.tile_pool(name="rsbp", bufs=2))

        ones_col = consts.tile([P, 1], dt.bfloat16, name="ones_col")
        ones_row = consts.tile([1, P], dt.bfloat16, name="ones_row")
        maskL = consts.tile([4, P], dt.bfloat16, name="maskL")
        maskR = consts.tile([4, P], dt.bfloat16, name="maskR")
        junk_w = consts.tile([P, P], dt.bfloat16, name="junk_w")
        junk_m = consts.tile([P, P], dt.bfloat16, name="junk_m")
        bqc_sb = consts.tile([P, KT], dt.bfloat16, name="bqc_sb")
        bvb_sb = consts.tile([1, D], dt.bfloat16, name="bvb_sb")
        w_sb = consts.tile([P, OT], dt.float32, name="w_sb")
        bv_rep = consts.tile([P, D], dt.float32, name="bv_rep")

        nc.vector.memset(ones_col[:], 1.0)
        nc.vector.memset(ones_row[:], 1.0)
        nc.vector.memset(junk_w[:], 0.0)
        nc.vector.memset(junk_m[:], 0.0)

        # big SBUF residents; each DMA below is one 1-2 MB op
        xTall = resid.tile([P, TCH * KT * FD], dt.bfloat16, name="xTall")
        wkall = resid.tile([P, KT * D], dt.bfloat16, name="wkall")
        wqall = resid.tile([P, KT * D], dt.bfloat16, name="wqall")
        wvall = resid.tile([P, KT * D], dt.bfloat16, name="wvall")
        A_sb = [resid.tile([P, D], dt.bfloat16, name=f"A{m}")
                for m in range(KT)]
        hT = [resid.tile([P, 2048], dt.bfloat16, name=f"hT{o}")
              for o in range(OT)]
        vv = [resid.tile([P, D], dt.bfloat16, name=f"v{t}") for t in range(NTT)]
        pT = [resid.tile([P, P], dt.bfloat16, name=f"pT{g}") for g in range(NTT)]

        def xT(k, lo, n):
            """columns [lo, lo+n) of d-stripe k of x.T (token index)."""
            c, j = divmod(lo, FD)
            assert j + n <= FD
            base = (KT * c + k) * FD + j
            return xTall[:, base:base + n]

        def wsl(wall, i, lo, n):
            return wall[:, D * i + lo:D * i + lo + n]

        # ---- DMA priority order. Ring A (scalar): wk half, wq half, wv.
        # Ring B (sync): tiny consts, wk half, wq half, x chunks 0..3.
        nc.sync.dma_start(bqc_sb[:], bqc_d[:])
        nc.sync.dma_start(bvb_sb[:], bvb_d[:])
        nc.sync.dma_start(maskL[:], mskl_d[:])
        nc.sync.dma_start(maskR[:], mskr_d[:])

        def load_w_half(eng, wall, w_d, hf):
            src = bass.AP(w_d, hf * 4 * P * D, [[D, P], [P * D, 4], [1, D]])
            eng.dma_start(wall[:, hf * 4 * D:(hf + 1) * 4 * D], src)

        load_w_half(nc.scalar, wkall, wk_d, 0)
        load_w_half(nc.sync, wkall, wk_d, 1)
        load_w_half(nc.scalar, wqall, wq_d, 0)
        load_w_half(nc.sync, wqall, wq_d, 1)
        src = bass.AP(wv_d, 0, [[D, P], [P * D, KT], [1, D]])
        nc.scalar.dma_start(wvall[:], src)
        for c in range(TCH):
            src = bass.AP(xt_d, c * D * FD, [[FD, P], [P * FD, KT], [1, FD]])
            nc.sync.dma_start(xTall[:, KT * FD * c:KT * FD * (c + 1)], src)

        with tc.tile_pool(name="projp", bufs=3, space="PSUM") as projp, \
             tc.tile_pool(name="simp", bufs=2, space="PSUM") as simp, \
             tc.tile_pool(name="avp", bufs=2, space="PSUM") as avp, \
             tc.tile_pool(name="lp", bufs=1, space="PSUM") as lp:

            # ---- HAM warm-up: junk matmuls from t~0 so the PE clock is
            # at 2.4 GHz when the first real matmuls arrive.
            if KWARM:
                wps = projp.tile([P, FD], dt.float32, name="pps")
                for _ in range(KWARM):
                    nc.tensor.matmul(wps[:, 0:P], junk_w[:], junk_m[:],
                                     start=True, stop=True)

            # ---- bv broadcast to all partitions via K=1 ones matmul
            for dh in range(2):
                ps = projp.tile([P, FD], dt.float32, name="pps")
                nc.tensor.matmul(ps[:], ones_row[:],
                                 bvb_sb[:, FD * dh:FD * dh + FD],
                                 start=True, stop=True)
                nc.scalar.copy(bv_rep[:, FD * dh:FD * dh + FD], ps[:])

            # ---- w = bq @ Wk, column m at a time (psum [P, OT])
            w_ps = lp.tile([P, OT], dt.float32, name="lps")
            for m in range(OT):
                for i in range(KT):
                    nc.tensor.matmul(w_ps[:, m:m + 1],
                                     wsl(wkall, i, P * m, P),
                                     bqc_sb[:, i:i + 1],
                                     start=(i == 0), stop=(i == KT - 1))
            nc.vector.tensor_copy(w_sb[:], w_ps[:])

            # ---- A = Wq^T @ Wk  (A[a, b] = sum_o Wq[o, a] Wk[o, b])
            for hf in range(2):
                for m in range(KT):
                    psA = projp.tile([P, FD], dt.float32, name="pps")
                    for i in range(KT):
                        nc.tensor.matmul(psA[:],
                                         wsl(wqall, i, P * m, P),
                                         wsl(wkall, i, FD * hf, FD),
                                         start=(i == 0), stop=(i == KT - 1))
                    nc.vector.tensor_copy(A_sb[m][:, FD * hf:FD * hf + FD],
                                          psA[:])

            def v_pass(t):
                pss = [projp.tile([P, FD], dt.float32, name="pps")
                       for _ in range(2)]
                for i in range(KT):
                    for dh in range(2):
                        nc.tensor.matmul(pss[dh][:],
                                         xT(i, P * t, P),
                                         wsl(wvall, i, FD * dh, FD),
                                         start=(i == 0), stop=(i == KT - 1))
                for dh in range(2):
                    nc.vector.tensor_copy(vv[t][:, FD * dh:FD * dh + FD],
                                          pss[dh][:])

            def av_group(t, eng):
                lps = lp.tile([P, 1], dt.float32, name="lps")
                nc.tensor.matmul(lps[:], pT[t][:], ones_col[:],
                                 start=True, stop=True)
                rsb = rsbp.tile([P, 1], dt.float32, name="rsb")
                nc.vector.reciprocal(rsb[:], lps[:])
                osb = outp.tile([P, D], dt.float32, name="osb")
                for dh in range(2):
                    avs = avp.tile([P, FD], dt.float32, name="avs")
                    nc.tensor.matmul(avs[:], pT[t][:],
                                     vv[t][:, FD * dh:FD * dh + FD],
                                     start=True, stop=True)
                    nc.vector.scalar_tensor_tensor(
                        osb[:, FD * dh:FD * dh + FD], avs[:], rsb[:],
                        bv_rep[:, FD * dh:FD * dh + FD],
                        ALU.mult, ALU.add)
                eng.dma_start(bass.AP(out_d, t * P * D, [[D, P], [1, D]]),
                              osb[:])

            for c in range(TCH):
                # v for this chunk (needs only x chunk c + Wv)
                for t in range(4 * c, 4 * c + 4):
                    v_pass(t)
                # h-pass for chunk c: h = x A + w, stored transposed
                for o in range(OT):
                    psH = projp.tile([P, FD], dt.float32, name="pps")
                    for i in range(KT):
                        nc.tensor.matmul(psH[:],
                                         A_sb[i][:, P * o:P * o + P],
                                         xT(i, FD * c, FD),
                                         start=(i == 0), stop=(i == KT - 1))
                    nc.scalar.activation(hT[o][:, FD * c:FD * c + FD],
                                         psH[:], AF.Identity,
                                         bias=w_sb[:, o:o + 1], scale=1.0)
                # simT + exp for this chunk's 4 groups
                for g in range(4 * c, 4 * c + 4):
                    sps = simp.tile([P, P], dt.float32, name="sps")
                    nc.tensor.matmul(sps[:], maskL[:], maskR[:],
                                     start=True, stop=False)
                    for kk in range(KT):
                        nc.tensor.matmul(sps[:],
                                         xT(kk, P * g, P),
                                         hT[kk][:, P * g:P * g + P],
                                         start=False, stop=(kk == KT - 1))
                    nc.scalar.activation(pT[g][:], sps[:], AF.Exp,
                                         bias=0.0, scale=scale)
                # attn@v for this chunk; out rides the idle HWDGE rings
                for t in range(4 * c, 4 * c + 4):
                    av_group(t, nc.sync if t % 2 == 0 else nc.scalar)

    nc.compile()
    return nc


def get_nc():
    if "nc" not in _CACHE:
        _CACHE["nc"] = _build_nc()
    return _CACHE["nc"]


def make_in_maps(x, Wq, bq, Wk, bk, Wv, bv):
    import ml_dtypes

    bf16 = ml_dtypes.bfloat16
    x = np.asarray(x, np.float32)
    wqn = np.ascontiguousarray(np.asarray(Wq, np.float32).astype(bf16))
    wkn = np.ascontiguousarray(np.asarray(Wk, np.float32).astype(bf16))
    wvt = np.ascontiguousarray(np.asarray(Wv, np.float32).astype(bf16).T)
    bqc = np.ascontiguousarray(
        np.asarray(bq, np.float32).astype(bf16).reshape(KT, P).T)
    bvb = np.asarray(bv, np.float32).reshape(1, D).astype(bf16)
    # maskL[a, i] = 1 iff i in 32-block a; maskR[a, j] = NEG unless j in a
    blk = (np.arange(P) // 32)[None, :] == np.arange(4)[:, None]
    mskl = blk.astype(bf16)
    mskr = np.where(blk, 0.0, NEG).astype(bf16)
    in_maps = []
    for c in range(8):
        b, h = divmod(c, 2)
        xs = x[b, 4096 * h:4096 * h + 4096]
        xs = xs.reshape(64, 64, D)[:, ::2, :].reshape(2048, D).astype(bf16)
        # x.T in chunk-major rows: row 1024*c + d = x.T[d, 512c:512c+512]
        xt = np.ascontiguousarray(
            xs.T.reshape(D, TCH, FD).transpose(1, 0, 2)).reshape(TCH * D, FD)
        in_maps.append({"xt": xt, "wqn": wqn, "wkn": wkn, "wvt": wvt,
                        "bqc": bqc, "bvb": bvb, "mskl": mskl, "mskr": mskr})
    return in_maps


def kernel(x, Wq, bq, Wk, bk, Wv, bv):
    from concourse.bass_utils import run_bass_kernel_spmd

    nc = get_nc()
    in_maps = make_in_maps(x, Wq, bq, Wk, bk, Wv, bv)
    res = run_bass_kernel_spmd(nc, in_maps, core_ids=list(range(8)))
    _CACHE["last_res"] = res
    out = np.empty((4, 4096, D), np.float32)
    for c in range(8):
        b, h = divmod(c, 2)
        out[b, 2048 * h:2048 * h + 2048] = res.results[c]["out"]
    return out


# revision 11
# speedup vs baseline: 1.2553x; 1.0236x over previous
"""Dilated segment attention on 8 Trainium2 NeuronCores (Bass/Tile).

Problem: x:[4,8192,1024] fp32. Per 64-token segment, rows ::2 are kept
(32 tokens), projected with Wq/Wk/Wv (+bias), and full-dim attention is
computed within each segment. Output: [4,4096,1024] fp32.

Sharding: data-parallel. Core c handles batch c//2, sequence half c%2 ->
2048 dilated tokens = 64 segments. No collectives.

Key algebraic restructure: softmax over keys cancels every term of
q_j.k_i that is constant in the key index i, so

  softmax_i(q_j . k_i) = softmax_i( x_j A x_i^T + x_i . w ),
  A = Wq^T Wk,  w = bq Wk        (bk drops out entirely).

The kernel computes A on-device once per core (1024^3 MACs, started as
soon as the 4 MB of Wq/Wk bf16 lands) and replaces BOTH the q and k
projections with a single h-projection h = x A + w; simT[i,j] = x_i.h_j.
This cuts projection matmul work from 3 passes to 2 (plus the cheap A).

Host prep is layout/dtype only: weights passed bf16 (native Wq/Wk for
the A matmul, Wv^T for the v pass), x passed dilated+transposed+bf16 in
chunk-major [4*1024, 512] layout. All SBUF residents use single big
tiles so each load is ONE 1-2 MB DMA op (descriptor-efficient), halves
split across the two HWDGE rings (sync + scalar) in priority order
wk -> wq -> x chunk 0 / wv -> x chunks 1-3.

Per-core pipeline (all matmuls bf16, fp32 PSUM):
  - warm-up junk matmuls keep the PE HAM at 2.4 GHz until real work
  - w = bq Wk (64 tiny matmuls, needs only Wk), A = Wq^T Wk (128
    matmuls, N=512) -> A in SBUF bf16
  - per chunk c: v-pass (x stationary, Wv^T moving); h-pass (A
    stationary, x.T moving) with w fused into the ACT psum->sbuf
    epilogue; simT per 4-segment group as one packed 128x128 matmul
    over 8 k-tiles plus a rank-4 mask matmul that puts -30000 on the
    off-diagonal 32x32 blocks so one full-tile ACT exp yields the
    block-diagonal p (off-diag underflows to exactly 0); attn@v and the
    softmax denominator l (ones-column matmul) per token tile; final
    out = psum_av * (1/l) + bv in one DVE scalar_tensor_tensor, written
    out on the (by then idle) HWDGE rings.
"""

import numpy as np

P = 128
D = 1024
KT = 8    # d tiles of 128
OT = 8    # d_out tiles of 128
NTT = 16  # token tiles of 128 (2048 tokens per core)
FD = 512  # matmul moving free dim / psum bank
TCH = 4   # token chunks of 512
NEG = -30000.0  # off-diagonal mask; exp(scale*(sim+NEG)) underflows to 0

_CACHE = {}


def _build_nc():
    import os
    from contextlib import ExitStack

    import concourse.bass as bass
    import concourse.mybir as mybir
    import concourse.tile as tile
    from concourse import bacc

    KWARM = int(os.environ.get("KWARM", "72"))

    dt = mybir.dt
    AF = mybir.ActivationFunctionType
    ALU = mybir.AluOpType

    nc = bacc.Bacc("TRN2", target_bir_lowering=False, debug=False,
                   enable_asserts=False)

    # x.T, dilated, bf16, chunk-major: row 1024*c + d holds x.T[d, 512c:...]
    xt_d = nc.dram_tensor("xt", [TCH * D, FD], dt.bfloat16,
                          kind="ExternalInput")
    wq_d = nc.dram_tensor("wqn", [D, D], dt.bfloat16, kind="ExternalInput")
    wk_d = nc.dram_tensor("wkn", [D, D], dt.bfloat16, kind="ExternalInput")
    wv_d = nc.dram_tensor("wvt", [D, D], dt.bfloat16, kind="ExternalInput")
    bqc_d = nc.dram_tensor("bqc", [P, KT], dt.bfloat16, kind="ExternalInput")
    bvb_d = nc.dram_tensor("bvb", [1, D], dt.bfloat16, kind="ExternalInput")
    mskl_d = nc.dram_tensor("mskl", [4, P], dt.bfloat16, kind="ExternalInput")
    mskr_d = nc.dram_tensor("mskr", [4, P], dt.bfloat16, kind="ExternalInput")
    out_d = nc.dram_tensor("out", [2048, D], dt.float32, kind="ExternalOutput")

    scale = float(D) ** -0.5

    with tile.TileContext(nc) as tc, ExitStack() as ctx:
        consts = ctx.enter_context(tc.tile_pool(name="consts", bufs=1))
        resid = ctx.enter_context(tc.tile_pool(name="resid", bufs=1))
        outp = ctx.enter_context(tc.tile_pool(name="outp", bufs=3))
        rsbp = ctx.enter_context(tc.tile_pool(name="rsbp", bufs=2))

        ones_col = consts.tile([P, 1], dt.bfloat16, name="ones_col")
        ones_row = consts.tile([1, P], dt.bfloat16, name="ones_row")
        maskL = consts.tile([4, P], dt.bfloat16, name="maskL")
        maskR = consts.tile([4, P], dt.bfloat16, name="maskR")
        junk_w = consts.tile([P, P], dt.bfloat16, name="junk_w")
        junk_m = consts.tile([P, P], dt.bfloat16, name="junk_m")
        bqc_sb = consts.tile([P, KT], dt.bfloat16, name="bqc_sb")
        bvb_sb = consts.tile([1, D], dt.bfloat16, name="bvb_sb")
        w_sb = consts.tile([P, OT], dt.float32, name="w_sb")
        bv_rep = consts.tile([P, D], dt.float32, name="bv_rep")

        nc.vector.memset(ones_col[:], 1.0)
        nc.vector.memset(ones_row[:], 1.0)
        nc.vector.memset(junk_w[:], 0.0)
        nc.vector.memset(junk_m[:], 0.0)

        # big SBUF residents; each DMA below is one 1-2 MB op
        xTall = resid.tile([P, TCH * KT * FD], dt.bfloat16, name="xTall")
        wkall = resid.tile([P, KT * D], dt.bfloat16, name="wkall")
        wqall = resid.tile([P, KT * D], dt.bfloat16, name="wqall")
        wvall = resid.tile([P, KT * D], dt.bfloat16, name="wvall")
        A_sb = [resid.tile([P, D], dt.bfloat16, name=f"A{m}")
                for m in range(KT)]
        hT = [resid.tile([P, 2048], dt.bfloat16, name=f"hT{o}")
              for o in range(OT)]
        vv = [resid.tile([P, D], dt.bfloat16, name=f"v{t}") for t in range(NTT)]
        pT = [resid.tile([P, P], dt.bfloat16, name=f"pT{g}") for g in range(NTT)]

        def xT(k, lo, n):
            """columns [lo, lo+n) of d-stripe k of x.T (token index)."""
            c, j = divmod(lo, FD)
            assert j + n <= FD
            base = (KT * c + k) * FD + j
            return xTall[:, base:base + n]

        def wsl(wall, i, lo, n):
            return wall[:, D * i + lo:D * i + lo + n]

        # ---- DMA priority order. Ring A (scalar): wk half, wq half, wv.
        # Ring B (sync): tiny consts, wk half, wq half, x chunks 0..3.
        nc.sync.dma_start(bqc_sb[:], bqc_d[:])
        nc.sync.dma_start(bvb_sb[:], bvb_d[:])
        nc.sync.dma_start(maskL[:], mskl_d[:])
        nc.sync.dma_start(maskR[:], mskr_d[:])

        def load_w_half(eng, wall, w_d, hf):
            src = bass.AP(w_d, hf * 4 * P * D, [[D, P], [P * D, 4], [1, D]])
            eng.dma_start(wall[:, hf * 4 * D:(hf + 1) * 4 * D], src)

        load_w_half(nc.scalar, wkall, wk_d, 0)
        load_w_half(nc.sync, wqall, wq_d, 0)
        load_w_half(nc.scalar, wqall, wq_d, 1)
        load_w_half(nc.sync, wkall, wk_d, 1)
        src = bass.AP(wv_d, 0, [[D, P], [P * D, KT], [1, D]])
        nc.scalar.dma_start(wvall[:], src)
        for c in range(TCH):
            src = bass.AP(xt_d, c * D * FD, [[FD, P], [P * FD, KT], [1, FD]])
            nc.sync.dma_start(xTall[:, KT * FD * c:KT * FD * (c + 1)], src)

        with tc.tile_pool(name="projp", bufs=3, space="PSUM") as projp, \
             tc.tile_pool(name="simp", bufs=2, space="PSUM") as simp, \
             tc.tile_pool(name="avp", bufs=2, space="PSUM") as avp, \
             tc.tile_pool(name="lp", bufs=1, space="PSUM") as lp:

            # ---- HAM warm-up: junk matmuls from t~0 so the PE clock is
            # at 2.4 GHz when the first real matmuls arrive.
            if KWARM:
                wps = projp.tile([P, FD], dt.float32, name="pps")
                for _ in range(KWARM):
                    nc.tensor.matmul(wps[:, 0:P], junk_w[:], junk_m[:],
                                     start=True, stop=True)

            # ---- bv broadcast to all partitions via K=1 ones matmul
            for dh in range(2):
                ps = projp.tile([P, FD], dt.float32, name="pps")
                nc.tensor.matmul(ps[:], ones_row[:],
                                 bvb_sb[:, FD * dh:FD * dh + FD],
                                 start=True, stop=True)
                nc.scalar.copy(bv_rep[:, FD * dh:FD * dh + FD], ps[:])

            # ---- A = Wq^T @ Wk  (A[a, b] = sum_o Wq[o, a] Wk[o, b]).
            # i-OUTER with 8 simultaneous accumulators (one psum bank each,
            # borrowed across all four pools = exactly 8 banks): the first
            # 4 i-steps contract the first halves of Wq/Wk, so the sweep
            # starts as soon as those 2 MB land instead of waiting for all
            # 4 MB.  w = bq @ Wk slots between the two half-sweeps, giving
            # the DVE time to evacuate sweep 0 before its banks are reused.
            def a_sweep(hf):
                acc = [projp.tile([P, FD], dt.float32, name="pps")
                       for _ in range(3)]
                acc += [simp.tile([P, FD], dt.float32, name="sps")
                        for _ in range(2)]
                acc += [avp.tile([P, FD], dt.float32, name="avs")
                        for _ in range(2)]
                acc += [lp.tile([P, FD], dt.float32, name="lps")]
                for i in range(KT):
                    for m in range(KT):
                        nc.tensor.matmul(acc[m][:],
                                         wsl(wqall, i, P * m, P),
                                         wsl(wkall, i, FD * hf, FD),
                                         start=(i == 0), stop=(i == KT - 1))
                for m in range(KT):
                    nc.vector.tensor_copy(A_sb[m][:, FD * hf:FD * hf + FD],
                                          acc[m][:])

            a_sweep(0)

            # ---- w = bq @ Wk, column m at a time (psum [P, OT])
            w_ps = lp.tile([P, OT], dt.float32, name="lps")
            for m in range(OT):
                for i in range(KT):
                    nc.tensor.matmul(w_ps[:, m:m + 1],
                                     wsl(wkall, i, P * m, P),
                                     bqc_sb[:, i:i + 1],
                                     start=(i == 0), stop=(i == KT - 1))
            nc.vector.tensor_copy(w_sb[:], w_ps[:])

            a_sweep(1)

            def v_pass(t):
                pss = [projp.tile([P, FD], dt.float32, name="pps")
                       for _ in range(2)]
                for i in range(KT):
                    for dh in range(2):
                        nc.tensor.matmul(pss[dh][:],
                                         xT(i, P * t, P),
                                         wsl(wvall, i, FD * dh, FD),
                                         start=(i == 0), stop=(i == KT - 1))
                for dh in range(2):
                    nc.vector.tensor_copy(vv[t][:, FD * dh:FD * dh + FD],
                                          pss[dh][:])

            def av_group(t, eng):
                lps = lp.tile([P, 1], dt.float32, name="lps")
                nc.tensor.matmul(lps[:], pT[t][:], ones_col[:],
                                 start=True, stop=True)
                rsb = rsbp.tile([P, 1], dt.float32, name="rsb")
                nc.vector.reciprocal(rsb[:], lps[:])
                osb = outp.tile([P, D], dt.float32, name="osb")
                for dh in range(2):
                    avs = avp.tile([P, FD], dt.float32, name="avs")
                    nc.tensor.matmul(avs[:], pT[t][:],
                                     vv[t][:, FD * dh:FD * dh + FD],
                                     start=True, stop=True)
                    nc.vector.scalar_tensor_tensor(
                        osb[:, FD * dh:FD * dh + FD], avs[:], rsb[:],
                        bv_rep[:, FD * dh:FD * dh + FD],
                        ALU.mult, ALU.add)
                    eng.dma_start(
                        bass.AP(out_d, t * P * D + FD * dh,
                                [[D, P], [1, FD]]),
                        osb[:, FD * dh:FD * dh + FD])

            for c in range(TCH):
                # v for this chunk (needs only x chunk c + Wv)
                for t in range(4 * c, 4 * c + 4):
                    v_pass(t)
                # h-pass for chunk c: h = x A + w, stored transposed
                for o in range(OT):
                    psH = projp.tile([P, FD], dt.float32, name="pps")
                    for i in range(KT):
                        nc.tensor.matmul(psH[:],
                                         A_sb[i][:, P * o:P * o + P],
                                         xT(i, FD * c, FD),
                                         start=(i == 0), stop=(i == KT - 1))
                    nc.scalar.activation(hT[o][:, FD * c:FD * c + FD],
                                         psH[:], AF.Identity,
                                         bias=w_sb[:, o:o + 1], scale=1.0)
                # simT + exp for this chunk's 4 groups
                for g in range(4 * c, 4 * c + 4):
                    sps = simp.tile([P, P], dt.float32, name="sps")
                    nc.tensor.matmul(sps[:], maskL[:], maskR[:],
                                     start=True, stop=False)
                    for kk in range(KT):
                        nc.tensor.matmul(sps[:],
                                         xT(kk, P * g, P),
                                         hT[kk][:, P * g:P * g + P],
                                         start=False, stop=(kk == KT - 1))
                    nc.scalar.activation(pT[g][:], sps[:], AF.Exp,
                                         bias=0.0, scale=scale)
                # attn@v for this chunk; out rides the idle HWDGE rings
                for t in range(4 * c, 4 * c + 4):
                    av_group(t, nc.sync if t % 2 == 0 else nc.scalar)

    nc.compile()
    return nc


def get_nc():
    if "nc" not in _CACHE:
        _CACHE["nc"] = _build_nc()
    return _CACHE["nc"]


def make_in_maps(x, Wq, bq, Wk, bk, Wv, bv):
    import ml_dtypes

    bf16 = ml_dtypes.bfloat16
    x = np.asarray(x, np.float32)
    wqn = np.ascontiguousarray(np.asarray(Wq, np.float32).astype(bf16))
    wkn = np.ascontiguousarray(np.asarray(Wk, np.float32).astype(bf16))
    wvt = np.ascontiguousarray(np.asarray(Wv, np.float32).astype(bf16).T)
    bqc = np.ascontiguousarray(
        np.asarray(bq, np.float32).astype(bf16).reshape(KT, P).T)
    bvb = np.asarray(bv, np.float32).reshape(1, D).astype(bf16)
    # maskL[a, i] = 1 iff i in 32-block a; maskR[a, j] = NEG unless j in a
    blk = (np.arange(P) // 32)[None, :] == np.arange(4)[:, None]
    mskl = blk.astype(bf16)
    mskr = np.where(blk, 0.0, NEG).astype(bf16)
    in_maps = []
    for c in range(8):
        b, h = divmod(c, 2)
        xs = x[b, 4096 * h:4096 * h + 4096]
        xs = xs.reshape(64, 64, D)[:, ::2, :].reshape(2048, D).astype(bf16)
        # x.T in chunk-major rows: row 1024*c + d = x.T[d, 512c:512c+512]
        xt = np.ascontiguousarray(
            xs.T.reshape(D, TCH, FD).transpose(1, 0, 2)).reshape(TCH * D, FD)
        in_maps.append({"xt": xt, "wqn": wqn, "wkn": wkn, "wvt": wvt,
                        "bqc": bqc, "bvb": bvb, "mskl": mskl, "mskr": mskr})
    return in_maps


def kernel(x, Wq, bq, Wk, bk, Wv, bv):
    from concourse.bass_utils import run_bass_kernel_spmd

    nc = get_nc()
    in_maps = make_in_maps(x, Wq, bq, Wk, bk, Wv, bv)
    res = run_bass_kernel_spmd(nc, in_maps, core_ids=list(range(8)))
    _CACHE["last_res"] = res
    out = np.empty((4, 4096, D), np.float32)
    for c in range(8):
        b, h = divmod(c, 2)
        out[b, 2048 * h:2048 * h + 2048] = res.results[c]["out"]
    return out


# revision 13
# speedup vs baseline: 1.2860x; 1.0244x over previous
"""Dilated segment attention on 8 Trainium2 NeuronCores (Bass/Tile).

Problem: x:[4,8192,1024] fp32. Per 64-token segment, rows ::2 are kept
(32 tokens), projected with Wq/Wk/Wv (+bias), and full-dim attention is
computed within each segment. Output: [4,4096,1024] fp32.

Sharding: data-parallel. Core c handles batch c//2, sequence half c%2 ->
2048 dilated tokens = 64 segments. No collectives.

Key algebraic restructure: softmax over keys cancels every term of
q_j.k_i that is constant in the key index i, so

  softmax_i(q_j . k_i) = softmax_i( x_j A x_i^T + x_i . w ),
  A = Wq^T Wk,  w = bq Wk        (bk drops out entirely).

The kernel computes A on-device once per core (1024^3 MACs, started as
soon as the 4 MB of Wq/Wk bf16 lands) and replaces BOTH the q and k
projections with a single h-projection h = x A + w; simT[i,j] = x_i.h_j.
This cuts projection matmul work from 3 passes to 2 (plus the cheap A).

Host prep is layout/dtype only: weights passed bf16 (native Wq/Wk for
the A matmul, Wv^T for the v pass), x passed dilated+transposed+bf16 in
chunk-major [4*1024, 512] layout. All SBUF residents use single big
tiles so each load is ONE 1-2 MB DMA op (descriptor-efficient), halves
split across the two HWDGE rings (sync + scalar) in priority order
wk -> wq -> x chunk 0 / wv -> x chunks 1-3.

Per-core pipeline (all matmuls bf16, fp32 PSUM):
  - warm-up junk matmuls keep the PE HAM at 2.4 GHz until real work
  - w = bq Wk (64 tiny matmuls, needs only Wk), A = Wq^T Wk (128
    matmuls, N=512) -> A in SBUF bf16
  - per chunk c: v-pass (x stationary, Wv^T moving); h-pass (A
    stationary, x.T moving) with w fused into the ACT psum->sbuf
    epilogue; simT per 4-segment group as one packed 128x128 matmul
    over 8 k-tiles plus a rank-4 mask matmul that puts -30000 on the
    off-diagonal 32x32 blocks so one full-tile ACT exp yields the
    block-diagonal p (off-diag underflows to exactly 0); attn@v and the
    softmax denominator l (ones-column matmul) per token tile; final
    out = psum_av * (1/l) + bv in one DVE scalar_tensor_tensor, written
    out on the (by then idle) HWDGE rings.
"""

import numpy as np

P = 128
D = 1024
KT = 8    # d tiles of 128
OT = 8    # d_out tiles of 128
NTT = 16  # token tiles of 128 (2048 tokens per core)
FD = 512  # matmul moving free dim / psum bank
TCH = 4   # token chunks of 512
NEG = -30000.0  # off-diagonal mask; exp(scale*(sim+NEG)) underflows to 0

_CACHE = {}


def _build_nc():
    import os
    from contextlib import ExitStack

    import concourse.bass as bass
    import concourse.mybir as mybir
    import concourse.tile as tile
    from concourse import bacc

    KWARM = int(os.environ.get("KWARM", "56"))

    dt = mybir.dt
    AF = mybir.ActivationFunctionType
    ALU = mybir.AluOpType

    nc = bacc.Bacc("TRN2", target_bir_lowering=False, debug=False,
                   enable_asserts=False)

    # x.T, dilated, bf16, chunk-major: row 1024*c + d holds x.T[d, 512c:...]
    xt_d = nc.dram_tensor("xt", [TCH * D, FD], dt.bfloat16,
                          kind="ExternalInput")
    wq_d = nc.dram_tensor("wqn", [D, D], dt.bfloat16, kind="ExternalInput")
    wk_d = nc.dram_tensor("wkn", [D, D], dt.bfloat16, kind="ExternalInput")
    wv_d = nc.dram_tensor("wvt", [D, D], dt.bfloat16, kind="ExternalInput")
    bqc_d = nc.dram_tensor("bqc", [P, KT], dt.bfloat16, kind="ExternalInput")
    bvb_d = nc.dram_tensor("bvb", [1, D], dt.bfloat16, kind="ExternalInput")
    mskl_d = nc.dram_tensor("mskl", [4, P], dt.bfloat16, kind="ExternalInput")
    mskr_d = nc.dram_tensor("mskr", [4, P], dt.bfloat16, kind="ExternalInput")
    out_d = nc.dram_tensor("out", [2048, D], dt.float32, kind="ExternalOutput")

    scale = float(D) ** -0.5

    with tile.TileContext(nc) as tc, ExitStack() as ctx:
        consts = ctx.enter_context(tc.tile_pool(name="consts", bufs=1))
        resid = ctx.enter_context(tc.tile_pool(name="resid", bufs=1))
        outp = ctx.enter_context(tc.tile_pool(name="outp", bufs=3))
        rsbp = ctx.enter_context(tc.tile_pool(name="rsbp", bufs=2))

        ones_col = consts.tile([P, 1], dt.bfloat16, name="ones_col")
        ones_row = consts.tile([1, P], dt.bfloat16, name="ones_row")
        maskL = consts.tile([4, P], dt.bfloat16, name="maskL")
        maskR = consts.tile([4, P], dt.bfloat16, name="maskR")
        junk_w = consts.tile([P, P], dt.bfloat16, name="junk_w")
        junk_m = consts.tile([P, P], dt.bfloat16, name="junk_m")
        bqc_sb = consts.tile([P, KT], dt.bfloat16, name="bqc_sb")
        bvb_sb = consts.tile([1, D], dt.bfloat16, name="bvb_sb")
        w_sb = consts.tile([P, OT], dt.float32, name="w_sb")
        bv_rep = consts.tile([P, D], dt.float32, name="bv_rep")

        nc.vector.memset(ones_col[:], 1.0)
        nc.vector.memset(ones_row[:], 1.0)
        nc.vector.memset(junk_w[:], 0.0)
        nc.vector.memset(junk_m[:], 0.0)

        # big SBUF residents; each DMA below is one 1-2 MB op
        xTall = resid.tile([P, TCH * KT * FD], dt.bfloat16, name="xTall")
        wkall = resid.tile([P, KT * D], dt.bfloat16, name="wkall")
        wqall = resid.tile([P, KT * D], dt.bfloat16, name="wqall")
        wvall = resid.tile([P, KT * D], dt.bfloat16, name="wvall")
        A_sb = [resid.tile([P, D], dt.bfloat16, name=f"A{m}")
                for m in range(KT)]
        hT = [resid.tile([P, 2048], dt.bfloat16, name=f"hT{o}")
              for o in range(OT)]
        vv = [resid.tile([P, D], dt.bfloat16, name=f"v{t}") for t in range(NTT)]
        pT = [resid.tile([P, P], dt.bfloat16, name=f"pT{g}") for g in range(NTT)]

        def xT(k, lo, n):
            """columns [lo, lo+n) of d-stripe k of x.T (token index)."""
            c, j = divmod(lo, FD)
            assert j + n <= FD
            base = (KT * c + k) * FD + j
            return xTall[:, base:base + n]

        def wsl(wall, i, lo, n):
            return wall[:, D * i + lo:D * i + lo + n]

        # ---- DMA priority order. Tiny consts ride the idle SWDGE ring.
        # Wk/Wq go in 2-tile (256 KB) pieces, interleaved across the two
        # HWDGE rings in i order, so the i-outer A sweep starts as soon as
        # the first pair lands.  Then wv (scalar) and x chunks (sync).
        nc.gpsimd.dma_start(bqc_sb[:], bqc_d[:])
        nc.gpsimd.dma_start(bvb_sb[:], bvb_d[:])
        nc.gpsimd.dma_start(maskL[:], mskl_d[:])
        nc.gpsimd.dma_start(maskR[:], mskr_d[:])

        def load_w_pair(eng, wall, w_d, pr):
            src = bass.AP(w_d, pr * 2 * P * D, [[D, P], [P * D, 2], [1, D]])
            eng.dma_start(wall[:, pr * 2 * D:(pr + 1) * 2 * D], src)

        for pr in range(4):
            load_w_pair(nc.scalar, wkall, wk_d, pr)
            load_w_pair(nc.sync, wqall, wq_d, pr)
        src = bass.AP(wv_d, 0, [[D, P], [P * D, KT], [1, D]])
        nc.scalar.dma_start(wvall[:], src)
        for c in range(TCH):
            src = bass.AP(xt_d, c * D * FD, [[FD, P], [P * FD, KT], [1, FD]])
            nc.sync.dma_start(xTall[:, KT * FD * c:KT * FD * (c + 1)], src)

        with tc.tile_pool(name="projp", bufs=3, space="PSUM") as projp, \
             tc.tile_pool(name="simp", bufs=2, space="PSUM") as simp, \
             tc.tile_pool(name="avp", bufs=2, space="PSUM") as avp, \
             tc.tile_pool(name="lp", bufs=1, space="PSUM") as lp:

            # ---- HAM warm-up: junk matmuls from t~0 so the PE clock is
            # at 2.4 GHz when the first real matmuls arrive.
            if KWARM:
                wps = projp.tile([P, FD], dt.float32, name="pps")
                for _ in range(KWARM):
                    nc.tensor.matmul(wps[:, 0:P], junk_w[:], junk_m[:],
                                     start=True, stop=True)

            # ---- bv broadcast to all partitions via K=1 ones matmul
            for dh in range(2):
                ps = projp.tile([P, FD], dt.float32, name="pps")
                nc.tensor.matmul(ps[:], ones_row[:],
                                 bvb_sb[:, FD * dh:FD * dh + FD],
                                 start=True, stop=True)
                nc.scalar.copy(bv_rep[:, FD * dh:FD * dh + FD], ps[:])

            # ---- A = Wq^T @ Wk  (A[a, b] = sum_o Wq[o, a] Wk[o, b]).
            # i-OUTER with 8 simultaneous accumulators (one psum bank each,
            # borrowed across all four pools = exactly 8 banks): the first
            # 4 i-steps contract the first halves of Wq/Wk, so the sweep
            # starts as soon as those 2 MB land instead of waiting for all
            # 4 MB.  w = bq @ Wk slots between the two half-sweeps, giving
            # the DVE time to evacuate sweep 0 before its banks are reused.
            def a_sweep(hf):
                acc = [projp.tile([P, FD], dt.float32, name="pps")
                       for _ in range(3)]
                acc += [simp.tile([P, FD], dt.float32, name="sps")
                        for _ in range(2)]
                acc += [avp.tile([P, FD], dt.float32, name="avs")
                        for _ in range(2)]
                acc += [lp.tile([P, FD], dt.float32, name="lps")]
                for i in range(KT):
                    for m in range(KT):
                        nc.tensor.matmul(acc[m][:],
                                         wsl(wqall, i, P * m, P),
                                         wsl(wkall, i, FD * hf, FD),
                                         start=(i == 0), stop=(i == KT - 1))
                for m in range(KT):
                    nc.vector.tensor_copy(A_sb[m][:, FD * hf:FD * hf + FD],
                                          acc[m][:])

            a_sweep(0)

            # ---- w = bq @ Wk, column m at a time (psum [P, OT])
            w_ps = lp.tile([P, OT], dt.float32, name="lps")
            for m in range(OT):
                for i in range(KT):
                    nc.tensor.matmul(w_ps[:, m:m + 1],
                                     wsl(wkall, i, P * m, P),
                                     bqc_sb[:, i:i + 1],
                                     start=(i == 0), stop=(i == KT - 1))
            nc.vector.tensor_copy(w_sb[:], w_ps[:])

            a_sweep(1)

            def v_pass(t):
                pss = [projp.tile([P, FD], dt.float32, name="pps")
                       for _ in range(2)]
                for i in range(KT):
                    for dh in range(2):
                        nc.tensor.matmul(pss[dh][:],
                                         xT(i, P * t, P),
                                         wsl(wvall, i, FD * dh, FD),
                                         start=(i == 0), stop=(i == KT - 1))
                for dh in range(2):
                    nc.vector.tensor_copy(vv[t][:, FD * dh:FD * dh + FD],
                                          pss[dh][:])

            def av_group(t, eng):
                lps = lp.tile([P, 1], dt.float32, name="lps")
                nc.tensor.matmul(lps[:], pT[t][:], ones_col[:],
                                 start=True, stop=True)
                rsb = rsbp.tile([P, 1], dt.float32, name="rsb")
                nc.vector.reciprocal(rsb[:], lps[:])
                osb = outp.tile([P, D], dt.float32, name="osb")
                for dh in range(2):
                    avs = avp.tile([P, FD], dt.float32, name="avs")
                    nc.tensor.matmul(avs[:], pT[t][:],
                                     vv[t][:, FD * dh:FD * dh + FD],
                                     start=True, stop=True)
                    nc.vector.scalar_tensor_tensor(
                        osb[:, FD * dh:FD * dh + FD], avs[:], rsb[:],
                        bv_rep[:, FD * dh:FD * dh + FD],
                        ALU.mult, ALU.add)
                    eng.dma_start(
                        bass.AP(out_d, t * P * D + FD * dh,
                                [[D, P], [1, FD]]),
                        osb[:, FD * dh:FD * dh + FD])

            for c in range(TCH):
                # v for this chunk (needs only x chunk c + Wv)
                for t in range(4 * c, 4 * c + 4):
                    v_pass(t)
                # h-pass for chunk c: h = x A + w, stored transposed
                for o in range(OT):
                    psH = projp.tile([P, FD], dt.float32, name="pps")
                    for i in range(KT):
                        nc.tensor.matmul(psH[:],
                                         A_sb[i][:, P * o:P * o + P],
                                         xT(i, FD * c, FD),
                                         start=(i == 0), stop=(i == KT - 1))
                    nc.scalar.activation(hT[o][:, FD * c:FD * c + FD],
                                         psH[:], AF.Identity,
                                         bias=w_sb[:, o:o + 1], scale=1.0)
                # simT + exp for this chunk's 4 groups
                for g in range(4 * c, 4 * c + 4):
                    sps = simp.tile([P, P], dt.float32, name="sps")
                    nc.tensor.matmul(sps[:], maskL[:], maskR[:],
                                     start=True, stop=False)
                    for kk in range(KT):
                        nc.tensor.matmul(sps[:],
                                         xT(kk, P * g, P),
                                         hT[kk][:, P * g:P * g + P],
                                         start=False, stop=(kk == KT - 1))
                    nc.scalar.activation(pT[g][:], sps[:], AF.Exp,
                                         bias=0.0, scale=scale)
                # attn@v for this chunk; out rides the idle HWDGE rings
                for t in range(4 * c, 4 * c + 4):
                    av_group(t, nc.sync if t % 2 == 0 else nc.scalar)

    nc.compile()
    return nc


def get_nc():
    if "nc" not in _CACHE:
        _CACHE["nc"] = _build_nc()
    return _CACHE["nc"]


def make_in_maps(x, Wq, bq, Wk, bk, Wv, bv):
    import ml_dtypes

    bf16 = ml_dtypes.bfloat16
    x = np.asarray(x, np.float32)
    wqn = np.ascontiguousarray(np.asarray(Wq, np.float32).astype(bf16))
    wkn = np.ascontiguousarray(np.asarray(Wk, np.float32).astype(bf16))
    wvt = np.ascontiguousarray(np.asarray(Wv, np.float32).astype(bf16).T)
    bqc = np.ascontiguousarray(
        np.asarray(bq, np.float32).astype(bf16).reshape(KT, P).T)
    bvb = np.asarray(bv, np.float32).reshape(1, D).astype(bf16)
    # maskL[a, i] = 1 iff i in 32-block a; maskR[a, j] = NEG unless j in a
    blk = (np.arange(P) // 32)[None, :] == np.arange(4)[:, None]
    mskl = blk.astype(bf16)
    mskr = np.where(blk, 0.0, NEG).astype(bf16)
    in_maps = []
    for c in range(8):
        b, h = divmod(c, 2)
        xs = x[b, 4096 * h:4096 * h + 4096]
        xs = xs.reshape(64, 64, D)[:, ::2, :].reshape(2048, D).astype(bf16)
        # x.T in chunk-major rows: row 1024*c + d = x.T[d, 512c:512c+512]
        xt = np.ascontiguousarray(
            xs.T.reshape(D, TCH, FD).transpose(1, 0, 2)).reshape(TCH * D, FD)
        in_maps.append({"xt": xt, "wqn": wqn, "wkn": wkn, "wvt": wvt,
                        "bqc": bqc, "bvb": bvb, "mskl": mskl, "mskr": mskr})
    return in_maps


def kernel(x, Wq, bq, Wk, bk, Wv, bv):
    from concourse.bass_utils import run_bass_kernel_spmd

    nc = get_nc()
    in_maps = make_in_maps(x, Wq, bq, Wk, bk, Wv, bv)
    res = run_bass_kernel_spmd(nc, in_maps, core_ids=list(range(8)))
    _CACHE["last_res"] = res
    out = np.empty((4, 4096, D), np.float32)
    for c in range(8):
        b, h = divmod(c, 2)
        out[b, 2048 * h:2048 * h + 2048] = res.results[c]["out"]
    return out


# revision 16
# speedup vs baseline: 1.3190x; 1.0257x over previous
"""Dilated segment attention on 8 Trainium2 NeuronCores (Bass/Tile).

Problem: x:[4,8192,1024] fp32. Per 64-token segment, rows ::2 are kept
(32 tokens), projected with Wq/Wk/Wv (+bias), and full-dim attention is
computed within each segment. Output: [4,4096,1024] fp32.

Sharding: data-parallel. Core c handles batch c//2, sequence half c%2 ->
2048 dilated tokens = 64 segments. No collectives.

Key algebraic restructure: softmax over keys cancels every term of
q_j.k_i that is constant in the key index i, so

  softmax_i(q_j . k_i) = softmax_i( x_j A x_i^T + x_i . w ),
  A = Wq^T Wk,  w = bq Wk        (bk drops out entirely).

The kernel computes A on-device once per core (1024^3 MACs, started as
soon as the 4 MB of Wq/Wk bf16 lands) and replaces BOTH the q and k
projections with a single h-projection h = x A + w; simT[i,j] = x_i.h_j.
This cuts projection matmul work from 3 passes to 2 (plus the cheap A).

Host prep is layout/dtype only: weights passed bf16 (native Wq/Wk for
the A matmul, Wv^T for the v pass), x passed dilated+transposed+bf16 in
chunk-major [4*1024, 512] layout. All SBUF residents use single big
tiles so each load is ONE 1-2 MB DMA op (descriptor-efficient), halves
split across the two HWDGE rings (sync + scalar) in priority order
wk -> wq -> x chunk 0 / wv -> x chunks 1-3.

Per-core pipeline (all matmuls bf16, fp32 PSUM):
  - warm-up junk matmuls keep the PE HAM at 2.4 GHz until real work
  - w = bq Wk (64 tiny matmuls, needs only Wk), A = Wq^T Wk (128
    matmuls, N=512) -> A in SBUF bf16
  - per chunk c: v-pass (x stationary, Wv^T moving); h-pass (A
    stationary, x.T moving) with w fused into the ACT psum->sbuf
    epilogue; simT per 4-segment group as one packed 128x128 matmul
    over 8 k-tiles plus a rank-4 mask matmul that puts -30000 on the
    off-diagonal 32x32 blocks so one full-tile ACT exp yields the
    block-diagonal p (off-diag underflows to exactly 0); attn@v and the
    softmax denominator l (ones-column matmul) per token tile; final
    out = psum_av * (1/l) + bv in one DVE scalar_tensor_tensor, written
    out on the (by then idle) HWDGE rings.
"""

import numpy as np

P = 128
D = 1024
KT = 8    # d tiles of 128
OT = 8    # d_out tiles of 128
NTT = 16  # token tiles of 128 (2048 tokens per core)
FD = 512  # matmul moving free dim / psum bank
TCH = 4   # token chunks of 512
NEG = -30000.0  # off-diagonal mask; exp(scale*(sim+NEG)) underflows to 0

_CACHE = {}


def _build_nc():
    import os
    from contextlib import ExitStack

    import concourse.bass as bass
    import concourse.mybir as mybir
    import concourse.tile as tile
    from concourse import bacc

    KWARM = int(os.environ.get("KWARM", "72"))

    dt = mybir.dt
    AF = mybir.ActivationFunctionType
    ALU = mybir.AluOpType

    nc = bacc.Bacc("TRN2", target_bir_lowering=False, debug=False,
                   enable_asserts=False)

    # x.T, dilated, bf16, chunk-major: row 1024*c + d holds x.T[d, 512c:...]
    xt_d = nc.dram_tensor("xt", [TCH * D, FD], dt.bfloat16,
                          kind="ExternalInput")
    wq_d = nc.dram_tensor("wqn", [D, D], dt.bfloat16, kind="ExternalInput")
    wk_d = nc.dram_tensor("wkn", [D, D], dt.bfloat16, kind="ExternalInput")
    wv_d = nc.dram_tensor("wvt", [D, D], dt.bfloat16, kind="ExternalInput")
    bqc_d = nc.dram_tensor("bqc", [P, KT], dt.bfloat16, kind="ExternalInput")
    bvb_d = nc.dram_tensor("bvb", [1, D], dt.bfloat16, kind="ExternalInput")
    mskl_d = nc.dram_tensor("mskl", [4, P], dt.bfloat16, kind="ExternalInput")
    mskr_d = nc.dram_tensor("mskr", [4, P], dt.bfloat16, kind="ExternalInput")
    out_d = nc.dram_tensor("out", [2048, D], dt.float32, kind="ExternalOutput")

    scale = float(D) ** -0.5

    with tile.TileContext(nc) as tc, ExitStack() as ctx:
        consts = ctx.enter_context(tc.tile_pool(name="consts", bufs=1))
        resid = ctx.enter_context(tc.tile_pool(name="resid", bufs=1))
        outp = ctx.enter_context(tc.tile_pool(name="outp", bufs=3))
        rsbp = ctx.enter_context(tc.tile_pool(name="rsbp", bufs=2))

        ones_col = consts.tile([P, 1], dt.bfloat16, name="ones_col")
        ones_row = consts.tile([1, P], dt.bfloat16, name="ones_row")
        maskL = consts.tile([4, P], dt.bfloat16, name="maskL")
        maskR = consts.tile([4, P], dt.bfloat16, name="maskR")
        junk_w = consts.tile([P, P], dt.bfloat16, name="junk_w")
        junk_m = consts.tile([P, P], dt.bfloat16, name="junk_m")
        bqc_sb = consts.tile([P, KT], dt.bfloat16, name="bqc_sb")
        bvb_sb = consts.tile([1, D], dt.bfloat16, name="bvb_sb")
        w_sb = consts.tile([P, OT], dt.float32, name="w_sb")
        bv_rep = consts.tile([P, D], dt.float32, name="bv_rep")

        nc.vector.memset(ones_col[:], 1.0)
        nc.vector.memset(ones_row[:], 1.0)
        nc.vector.memset(junk_w[:], 0.0)
        nc.vector.memset(junk_m[:], 0.0)

        # big SBUF residents; each DMA below is one 1-2 MB op
        xTall = resid.tile([P, TCH * KT * FD], dt.bfloat16, name="xTall")
        wkall = resid.tile([P, KT * D], dt.bfloat16, name="wkall")
        wqall = resid.tile([P, KT * D], dt.bfloat16, name="wqall")
        wvall = resid.tile([P, KT * D], dt.bfloat16, name="wvall")
        A_sb = [resid.tile([P, D], dt.bfloat16, name=f"A{m}")
                for m in range(KT)]
        hT = [resid.tile([P, 2048], dt.bfloat16, name=f"hT{o}")
              for o in range(OT)]
        vv = [resid.tile([P, D], dt.bfloat16, name=f"v{t}") for t in range(NTT)]
        pT = [resid.tile([P, P], dt.bfloat16, name=f"pT{g}") for g in range(NTT)]

        def xT(k, lo, n):
            """columns [lo, lo+n) of d-stripe k of x.T (token index)."""
            c, j = divmod(lo, FD)
            assert j + n <= FD
            base = (KT * c + k) * FD + j
            return xTall[:, base:base + n]

        def wsl(wall, i, lo, n):
            return wall[:, D * i + lo:D * i + lo + n]

        # ---- DMA priority order. Tiny consts ride the idle SWDGE ring.
        # Wk/Wq go in 2-tile (256 KB) pieces, interleaved across the two
        # HWDGE rings in i order, so the i-outer A sweep starts as soon as
        # the first pair lands.  Then wv (scalar) and x chunks (sync).
        nc.gpsimd.dma_start(bqc_sb[:], bqc_d[:])
        nc.gpsimd.dma_start(bvb_sb[:], bvb_d[:])
        nc.gpsimd.dma_start(maskL[:], mskl_d[:])
        nc.gpsimd.dma_start(maskR[:], mskr_d[:])

        def load_w_pair(eng, wall, w_d, pr):
            src = bass.AP(w_d, pr * 2 * P * D, [[D, P], [P * D, 2], [1, D]])
            eng.dma_start(wall[:, pr * 2 * D:(pr + 1) * 2 * D], src)

        for pr in range(4):
            load_w_pair(nc.scalar, wkall, wk_d, pr)
            load_w_pair(nc.sync, wqall, wq_d, pr)
        src = bass.AP(wv_d, 0, [[D, P], [P * D, KT], [1, D]])
        nc.scalar.dma_start(wvall[:], src)
        for c in range(TCH):
            src = bass.AP(xt_d, c * D * FD, [[FD, P], [P * FD, KT], [1, FD]])
            nc.sync.dma_start(xTall[:, KT * FD * c:KT * FD * (c + 1)], src)

        with tc.tile_pool(name="projp", bufs=3, space="PSUM") as projp, \
             tc.tile_pool(name="simp", bufs=2, space="PSUM") as simp, \
             tc.tile_pool(name="avp", bufs=2, space="PSUM") as avp, \
             tc.tile_pool(name="lp", bufs=1, space="PSUM") as lp:

            # ---- HAM warm-up: junk matmuls from t~0 so the PE clock is
            # at 2.4 GHz when the first real matmuls arrive.
            if KWARM:
                wps = projp.tile([P, FD], dt.float32, name="pps")
                for _ in range(KWARM):
                    nc.tensor.matmul(wps[:, 0:P], junk_w[:], junk_m[:],
                                     start=True, stop=True)

            # ---- A = Wq^T @ Wk  (A[a, b] = sum_o Wq[o, a] Wk[o, b]).
            # i-OUTER with 8 simultaneous accumulators (one psum bank each,
            # borrowed across all four pools = exactly 8 banks): the first
            # 4 i-steps contract the first halves of Wq/Wk, so the sweep
            # starts as soon as those 2 MB land instead of waiting for all
            # 4 MB.  w = bq @ Wk slots between the two half-sweeps, giving
            # the DVE time to evacuate sweep 0 before its banks are reused.
            def a_sweep(hf):
                acc = [projp.tile([P, FD], dt.float32, name="pps")
                       for _ in range(3)]
                acc += [simp.tile([P, FD], dt.float32, name="sps")
                        for _ in range(2)]
                acc += [avp.tile([P, FD], dt.float32, name="avs")
                        for _ in range(2)]
                acc += [lp.tile([P, FD], dt.float32, name="lps")]
                for i in range(KT):
                    for m in range(KT):
                        nc.tensor.matmul(acc[m][:],
                                         wsl(wqall, i, P * m, P),
                                         wsl(wkall, i, FD * hf, FD),
                                         start=(i == 0), stop=(i == KT - 1))
                for m in range(KT):
                    nc.vector.tensor_copy(A_sb[m][:, FD * hf:FD * hf + FD],
                                          acc[m][:])

            a_sweep(0)

            # ---- bv broadcast to all partitions via K=1 ones matmul
            # (needed only by the out epilogue; emitted here so the junk ->
            # sweep0 handoff isn't gated on the tiny bvb DMA)
            for dh in range(2):
                ps = projp.tile([P, FD], dt.float32, name="pps")
                nc.tensor.matmul(ps[:], ones_row[:],
                                 bvb_sb[:, FD * dh:FD * dh + FD],
                                 start=True, stop=True)
                nc.scalar.copy(bv_rep[:, FD * dh:FD * dh + FD], ps[:])

            # ---- w = bq @ Wk, column m at a time (psum [P, OT])
            w_ps = lp.tile([P, OT], dt.float32, name="lps")
            for m in range(OT):
                for i in range(KT):
                    nc.tensor.matmul(w_ps[:, m:m + 1],
                                     wsl(wkall, i, P * m, P),
                                     bqc_sb[:, i:i + 1],
                                     start=(i == 0), stop=(i == KT - 1))
            nc.vector.tensor_copy(w_sb[:], w_ps[:])

            a_sweep(1)

            def v_pass(t):
                pss = [projp.tile([P, FD], dt.float32, name="pps")
                       for _ in range(2)]
                for i in range(KT):
                    for dh in range(2):
                        nc.tensor.matmul(pss[dh][:],
                                         xT(i, P * t, P),
                                         wsl(wvall, i, FD * dh, FD),
                                         start=(i == 0), stop=(i == KT - 1))
                for dh in range(2):
                    nc.vector.tensor_copy(vv[t][:, FD * dh:FD * dh + FD],
                                          pss[dh][:])

            def av_group(t, eng):
                lps = lp.tile([P, 1], dt.float32, name="lps")
                nc.tensor.matmul(lps[:], pT[t][:], ones_col[:],
                                 start=True, stop=True)
                rsb = rsbp.tile([P, 1], dt.float32, name="rsb")
                nc.vector.reciprocal(rsb[:], lps[:])
                osb = outp.tile([P, D], dt.float32, name="osb")
                for dh in range(2):
                    avs = avp.tile([P, FD], dt.float32, name="avs")
                    nc.tensor.matmul(avs[:], pT[t][:],
                                     vv[t][:, FD * dh:FD * dh + FD],
                                     start=True, stop=True)
                    nc.vector.scalar_tensor_tensor(
                        osb[:, FD * dh:FD * dh + FD], avs[:], rsb[:],
                        bv_rep[:, FD * dh:FD * dh + FD],
                        ALU.mult, ALU.add)
                    eng.dma_start(
                        bass.AP(out_d, t * P * D + FD * dh,
                                [[D, P], [1, FD]]),
                        osb[:, FD * dh:FD * dh + FD])

            for c in range(TCH):
                # v for this chunk (needs only x chunk c + Wv)
                for t in range(4 * c, 4 * c + 4):
                    v_pass(t)
                # h-pass for chunk c: h = x A + w, stored transposed
                for o in range(OT):
                    psH = projp.tile([P, FD], dt.float32, name="pps")
                    for i in range(KT):
                        nc.tensor.matmul(psH[:],
                                         A_sb[i][:, P * o:P * o + P],
                                         xT(i, FD * c, FD),
                                         start=(i == 0), stop=(i == KT - 1))
                    nc.scalar.activation(hT[o][:, FD * c:FD * c + FD],
                                         psH[:], AF.Identity,
                                         bias=w_sb[:, o:o + 1], scale=1.0)
                # simT + exp for this chunk's 4 groups
                for g in range(4 * c, 4 * c + 4):
                    sps = simp.tile([P, P], dt.float32, name="sps")
                    nc.tensor.matmul(sps[:], maskL[:], maskR[:],
                                     start=True, stop=False)
                    for kk in range(KT):
                        nc.tensor.matmul(sps[:],
                                         xT(kk, P * g, P),
                                         hT[kk][:, P * g:P * g + P],
                                         start=False, stop=(kk == KT - 1))
                    nc.scalar.activation(pT[g][:], sps[:], AF.Exp,
                                         bias=0.0, scale=scale)
                # attn@v for this chunk; out rides the idle HWDGE rings
                for t in range(4 * c, 4 * c + 4):
                    av_group(t, nc.sync if t % 2 == 0 else nc.scalar)

    nc.compile()
    return nc


def get_nc():
    if "nc" not in _CACHE:
        _CACHE["nc"] = _build_nc()
    return _CACHE["nc"]


def make_in_maps(x, Wq, bq, Wk, bk, Wv, bv):
    import ml_dtypes

    bf16 = ml_dtypes.bfloat16
    x = np.asarray(x, np.float32)
    wqn = np.ascontiguousarray(np.asarray(Wq, np.float32).astype(bf16))
    wkn = np.ascontiguousarray(np.asarray(Wk, np.float32).astype(bf16))
    wvt = np.ascontiguousarray(np.asarray(Wv, np.float32).astype(bf16).T)
    bqc = np.ascontiguousarray(
        np.asarray(bq, np.float32).astype(bf16).reshape(KT, P).T)
    bvb = np.asarray(bv, np.float32).reshape(1, D).astype(bf16)
    # maskL[a, i] = 1 iff i in 32-block a; maskR[a, j] = NEG unless j in a
    blk = (np.arange(P) // 32)[None, :] == np.arange(4)[:, None]
    mskl = blk.astype(bf16)
    mskr = np.where(blk, 0.0, NEG).astype(bf16)
    in_maps = []
    for c in range(8):
        b, h = divmod(c, 2)
        xs = x[b, 4096 * h:4096 * h + 4096]
        xs = xs.reshape(64, 64, D)[:, ::2, :].reshape(2048, D).astype(bf16)
        # x.T in chunk-major rows: row 1024*c + d = x.T[d, 512c:512c+512]
        xt = np.ascontiguousarray(
            xs.T.reshape(D, TCH, FD).transpose(1, 0, 2)).reshape(TCH * D, FD)
        in_maps.append({"xt": xt, "wqn": wqn, "wkn": wkn, "wvt": wvt,
                        "bqc": bqc, "bvb": bvb, "mskl": mskl, "mskr": mskr})
    return in_maps


def kernel(x, Wq, bq, Wk, bk, Wv, bv):
    from concourse.bass_utils import run_bass_kernel_spmd

    nc = get_nc()
    in_maps = make_in_maps(x, Wq, bq, Wk, bk, Wv, bv)
    res = run_bass_kernel_spmd(nc, in_maps, core_ids=list(range(8)))
    _CACHE["last_res"] = res
    out = np.empty((4, 4096, D), np.float32)
    for c in range(8):
        b, h = divmod(c, 2)
        out[b, 2048 * h:2048 * h + 2048] = res.results[c]["out"]
    return out


# revision 20
# speedup vs baseline: 1.3383x; 1.0146x over previous
"""Dilated segment attention on 8 Trainium2 NeuronCores (Bass/Tile).

Problem: x:[4,8192,1024] fp32. Per 64-token segment, rows ::2 are kept
(32 tokens), projected with Wq/Wk/Wv (+bias), and full-dim attention is
computed within each segment. Output: [4,4096,1024] fp32.

Sharding: data-parallel. Core c handles batch c//2, sequence half c%2 ->
2048 dilated tokens = 64 segments. No collectives.

Key algebraic restructure: softmax over keys cancels every term of
q_j.k_i that is constant in the key index i, so

  softmax_i(q_j . k_i) = softmax_i( x_j A x_i^T + x_i . w ),
  A = Wq^T Wk,  w = bq Wk        (bk drops out entirely).

The kernel computes A on-device once per core (1024^3 MACs, started as
soon as the 4 MB of Wq/Wk bf16 lands) and replaces BOTH the q and k
projections with a single h-projection h = x A + w; simT[i,j] = x_i.h_j.
This cuts projection matmul work from 3 passes to 2 (plus the cheap A).

Host prep is layout/dtype only: weights passed bf16 (native Wq/Wk for
the A matmul, Wv^T for the v pass), x passed dilated+transposed+bf16 in
chunk-major [4*1024, 512] layout. All SBUF residents use single big
tiles so each load is ONE 1-2 MB DMA op (descriptor-efficient), halves
split across the two HWDGE rings (sync + scalar) in priority order
wk -> wq -> x chunk 0 / wv -> x chunks 1-3.

Per-core pipeline (all matmuls bf16, fp32 PSUM):
  - warm-up junk matmuls keep the PE HAM at 2.4 GHz until real work
  - w = bq Wk (64 tiny matmuls, needs only Wk), A = Wq^T Wk (128
    matmuls, N=512) -> A in SBUF bf16
  - per chunk c: v-pass (x stationary, Wv^T moving); h-pass (A
    stationary, x.T moving) with w fused into the ACT psum->sbuf
    epilogue; simT per 4-segment group as one packed 128x128 matmul
    over 8 k-tiles plus a rank-4 mask matmul that puts -30000 on the
    off-diagonal 32x32 blocks so one full-tile ACT exp yields the
    block-diagonal p (off-diag underflows to exactly 0); attn@v and the
    softmax denominator l (ones-column matmul) per token tile; final
    out = psum_av * (1/l) + bv in one DVE scalar_tensor_tensor, written
    out on the (by then idle) HWDGE rings.
"""

import numpy as np

P = 128
D = 1024
KT = 8    # d tiles of 128
OT = 8    # d_out tiles of 128
NTT = 16  # token tiles of 128 (2048 tokens per core)
FD = 512  # matmul moving free dim / psum bank
TCH = 4   # token chunks of 512
NEG = -30000.0  # off-diagonal mask; exp(scale*(sim+NEG)) underflows to 0

_CACHE = {}


def _build_nc():
    import os
    from contextlib import ExitStack

    import concourse.bass as bass
    import concourse.mybir as mybir
    import concourse.tile as tile
    from concourse import bacc

    KWARM = int(os.environ.get("KWARM", "60"))

    dt = mybir.dt
    AF = mybir.ActivationFunctionType
    ALU = mybir.AluOpType

    nc = bacc.Bacc("TRN2", target_bir_lowering=False, debug=False,
                   enable_asserts=False)

    # x.T, dilated, bf16, chunk-major: row 1024*c + d holds x.T[d, 512c:...]
    xt_d = nc.dram_tensor("xt", [TCH * D, FD], dt.bfloat16,
                          kind="ExternalInput")
    wq_d = nc.dram_tensor("wqn", [D, D], dt.bfloat16, kind="ExternalInput")
    wk_d = nc.dram_tensor("wkn", [D, D], dt.bfloat16, kind="ExternalInput")
    wv_d = nc.dram_tensor("wvt", [D, D], dt.bfloat16, kind="ExternalInput")
    bqc_d = nc.dram_tensor("bqc", [P, KT], dt.bfloat16, kind="ExternalInput")
    bvb_d = nc.dram_tensor("bvb", [1, D], dt.bfloat16, kind="ExternalInput")
    mskl_d = nc.dram_tensor("mskl", [4, P], dt.bfloat16, kind="ExternalInput")
    mskr_d = nc.dram_tensor("mskr", [4, P], dt.bfloat16, kind="ExternalInput")
    out_d = nc.dram_tensor("out", [2048, D], dt.float32, kind="ExternalOutput")

    scale = float(D) ** -0.5

    with tile.TileContext(nc) as tc, ExitStack() as ctx:
        consts = ctx.enter_context(tc.tile_pool(name="consts", bufs=1))
        resid = ctx.enter_context(tc.tile_pool(name="resid", bufs=1))
        outp = ctx.enter_context(tc.tile_pool(name="outp", bufs=3))
        rsbp = ctx.enter_context(tc.tile_pool(name="rsbp", bufs=2))

        ones_col = consts.tile([P, 1], dt.bfloat16, name="ones_col")
        ones_row = consts.tile([1, P], dt.bfloat16, name="ones_row")
        maskL = consts.tile([4, P], dt.bfloat16, name="maskL")
        maskR = consts.tile([4, P], dt.bfloat16, name="maskR")
        junk_w = consts.tile([P, P], dt.bfloat16, name="junk_w")
        junk_m = consts.tile([P, P], dt.bfloat16, name="junk_m")
        bqc_sb = consts.tile([P, KT], dt.bfloat16, name="bqc_sb")
        bvb_sb = consts.tile([1, D], dt.bfloat16, name="bvb_sb")
        w_sb = consts.tile([P, OT], dt.float32, name="w_sb")
        bv_rep = consts.tile([P, D], dt.float32, name="bv_rep")

        nc.vector.memset(ones_col[:], 1.0)
        nc.vector.memset(ones_row[:], 1.0)
        nc.vector.memset(junk_w[:], 0.0)
        nc.vector.memset(junk_m[:], 0.0)

        # big SBUF residents; each DMA below is one 1-2 MB op
        xTall = resid.tile([P, TCH * KT * FD], dt.bfloat16, name="xTall")
        wkall = resid.tile([P, KT * D], dt.bfloat16, name="wkall")
        wqall = resid.tile([P, KT * D], dt.bfloat16, name="wqall")
        wvall = resid.tile([P, KT * D], dt.bfloat16, name="wvall")
        A_sb = [resid.tile([P, D], dt.bfloat16, name=f"A{m}")
                for m in range(KT)]
        hT = [resid.tile([P, 2048], dt.bfloat16, name=f"hT{o}")
              for o in range(OT)]
        vv = [resid.tile([P, D], dt.bfloat16, name=f"v{t}") for t in range(NTT)]
        pT = [resid.tile([P, P], dt.bfloat16, name=f"pT{g}") for g in range(NTT)]

        def xT(k, lo, n):
            """columns [lo, lo+n) of d-stripe k of x.T (token index)."""
            c, j = divmod(lo, FD)
            assert j + n <= FD
            base = (KT * c + k) * FD + j
            return xTall[:, base:base + n]

        def wsl(wall, i, lo, n):
            return wall[:, D * i + lo:D * i + lo + n]

        # ---- DMA priority order. Tiny consts ride the idle SWDGE ring.
        # Wk/Wq go in 2-tile (256 KB) pieces, interleaved across the two
        # HWDGE rings in i order, so the i-outer A sweep starts as soon as
        # the first pair lands.  Then wv (scalar) and x chunks (sync).
        nc.gpsimd.dma_start(bqc_sb[:], bqc_d[:])
        nc.gpsimd.dma_start(bvb_sb[:], bvb_d[:])
        nc.gpsimd.dma_start(maskL[:], mskl_d[:])
        nc.gpsimd.dma_start(maskR[:], mskr_d[:])

        def load_w_tiles(eng, wall, w_d, lo, n):
            src = bass.AP(w_d, lo * P * D, [[D, P], [P * D, n], [1, D]])
            eng.dma_start(wall[:, lo * D:(lo + n) * D], src)

        # finest pieces first so the i-outer A sweep starts ASAP
        for i in range(2):
            load_w_tiles(nc.scalar, wkall, wk_d, i, 1)
            load_w_tiles(nc.sync, wqall, wq_d, i, 1)
        for pr in range(1, 4):
            load_w_tiles(nc.scalar, wkall, wk_d, 2 * pr, 2)
            load_w_tiles(nc.sync, wqall, wq_d, 2 * pr, 2)
        src = bass.AP(wv_d, 0, [[D, P], [P * D, KT], [1, D]])
        nc.scalar.dma_start(wvall[:], src)
        for c in range(TCH):
            src = bass.AP(xt_d, c * D * FD, [[FD, P], [P * FD, KT], [1, FD]])
            nc.sync.dma_start(xTall[:, KT * FD * c:KT * FD * (c + 1)], src)

        with tc.tile_pool(name="projp", bufs=3, space="PSUM") as projp, \
             tc.tile_pool(name="simp", bufs=2, space="PSUM") as simp, \
             tc.tile_pool(name="avp", bufs=2, space="PSUM") as avp, \
             tc.tile_pool(name="lp", bufs=1, space="PSUM") as lp:

            # ---- HAM warm-up: junk matmuls from t~0 so the PE clock is
            # at 2.4 GHz when the first real matmuls arrive.
            if KWARM:
                wps = projp.tile([P, FD], dt.float32, name="pps")
                for _ in range(KWARM):
                    nc.tensor.matmul(wps[:, 0:P], junk_w[:], junk_m[:],
                                     start=True, stop=True)

            # ---- A = Wq^T @ Wk  (A[a, b] = sum_o Wq[o, a] Wk[o, b]).
            # i-OUTER with 8 simultaneous accumulators (one psum bank each,
            # borrowed across all four pools = exactly 8 banks): the first
            # 4 i-steps contract the first halves of Wq/Wk, so the sweep
            # starts as soon as those 2 MB land instead of waiting for all
            # 4 MB.  w = bq @ Wk slots between the two half-sweeps, giving
            # the DVE time to evacuate sweep 0 before its banks are reused.
            def a_sweep(hf):
                acc = [projp.tile([P, FD], dt.float32, name="pps")
                       for _ in range(3)]
                acc += [simp.tile([P, FD], dt.float32, name="sps")
                        for _ in range(2)]
                acc += [avp.tile([P, FD], dt.float32, name="avs")
                        for _ in range(2)]
                acc += [lp.tile([P, FD], dt.float32, name="lps")]
                for i in range(KT):
                    for m in range(KT):
                        nc.tensor.matmul(acc[m][:],
                                         wsl(wqall, i, P * m, P),
                                         wsl(wkall, i, FD * hf, FD),
                                         start=(i == 0), stop=(i == KT - 1))
                for m in range(KT):
                    nc.vector.tensor_copy(A_sb[m][:, FD * hf:FD * hf + FD],
                                          acc[m][:])

            a_sweep(0)

            # ---- bv broadcast to all partitions via K=1 ones matmul
            # (needed only by the out epilogue; emitted here so the junk ->
            # sweep0 handoff isn't gated on the tiny bvb DMA)
            for dh in range(2):
                ps = projp.tile([P, FD], dt.float32, name="pps")
                nc.tensor.matmul(ps[:], ones_row[:],
                                 bvb_sb[:, FD * dh:FD * dh + FD],
                                 start=True, stop=True)
                nc.scalar.copy(bv_rep[:, FD * dh:FD * dh + FD], ps[:])

            # ---- w = bq @ Wk, column m at a time (psum [P, OT])
            w_ps = lp.tile([P, OT], dt.float32, name="lps")
            for m in range(OT):
                for i in range(KT):
                    nc.tensor.matmul(w_ps[:, m:m + 1],
                                     wsl(wkall, i, P * m, P),
                                     bqc_sb[:, i:i + 1],
                                     start=(i == 0), stop=(i == KT - 1))
            nc.vector.tensor_copy(w_sb[:], w_ps[:])

            a_sweep(1)

            def v_pass(t):
                pss = [projp.tile([P, FD], dt.float32, name="pps")
                       for _ in range(2)]
                for i in range(KT):
                    for dh in range(2):
                        nc.tensor.matmul(pss[dh][:],
                                         xT(i, P * t, P),
                                         wsl(wvall, i, FD * dh, FD),
                                         start=(i == 0), stop=(i == KT - 1))
                for dh in range(2):
                    nc.vector.tensor_copy(vv[t][:, FD * dh:FD * dh + FD],
                                          pss[dh][:])

            def av_group(t, eng):
                lps = lp.tile([P, 1], dt.float32, name="lps")
                nc.tensor.matmul(lps[:], pT[t][:], ones_col[:],
                                 start=True, stop=True)
                rsb = rsbp.tile([P, 1], dt.float32, name="rsb")
                nc.vector.reciprocal(rsb[:], lps[:])
                osb = outp.tile([P, D], dt.float32, name="osb")
                for dh in range(2):
                    avs = avp.tile([P, FD], dt.float32, name="avs")
                    nc.tensor.matmul(avs[:], pT[t][:],
                                     vv[t][:, FD * dh:FD * dh + FD],
                                     start=True, stop=True)
                    nc.vector.scalar_tensor_tensor(
                        osb[:, FD * dh:FD * dh + FD], avs[:], rsb[:],
                        bv_rep[:, FD * dh:FD * dh + FD],
                        ALU.mult, ALU.add)
                    eng.dma_start(
                        bass.AP(out_d, t * P * D + FD * dh,
                                [[D, P], [1, FD]]),
                        osb[:, FD * dh:FD * dh + FD])

            for c in range(TCH):
                # v for this chunk (needs only x chunk c + Wv).  The last
                # chunk's v tiles are instead interleaved with its attn@v
                # below, so the final STT/out epilogues hide behind v
                # matmuls instead of bunching DVE-bound at the very end.
                if c < TCH - 1:
                    for t in range(4 * c, 4 * c + 4):
                        v_pass(t)
                # h-pass for chunk c: h = x A + w, stored transposed
                for o in range(OT):
                    psH = projp.tile([P, FD], dt.float32, name="pps")
                    for i in range(KT):
                        nc.tensor.matmul(psH[:],
                                         A_sb[i][:, P * o:P * o + P],
                                         xT(i, FD * c, FD),
                                         start=(i == 0), stop=(i == KT - 1))
                    nc.scalar.activation(hT[o][:, FD * c:FD * c + FD],
                                         psH[:], AF.Identity,
                                         bias=w_sb[:, o:o + 1], scale=1.0)
                # simT + exp for this chunk's 4 groups
                for g in range(4 * c, 4 * c + 4):
                    sps = simp.tile([P, P], dt.float32, name="sps")
                    nc.tensor.matmul(sps[:], maskL[:], maskR[:],
                                     start=True, stop=False)
                    for kk in range(KT):
                        nc.tensor.matmul(sps[:],
                                         xT(kk, P * g, P),
                                         hT[kk][:, P * g:P * g + P],
                                         start=False, stop=(kk == KT - 1))
                    nc.scalar.activation(pT[g][:], sps[:], AF.Exp,
                                         bias=0.0, scale=scale)
                # attn@v for this chunk; out rides the idle HWDGE rings
                for t in range(4 * c, 4 * c + 4):
                    if c == TCH - 1:
                        v_pass(t)
                    av_group(t, nc.sync if t % 2 == 0 else nc.scalar)

    nc.compile()
    return nc


def get_nc():
    if "nc" not in _CACHE:
        _CACHE["nc"] = _build_nc()
    return _CACHE["nc"]


def make_in_maps(x, Wq, bq, Wk, bk, Wv, bv):
    import ml_dtypes

    bf16 = ml_dtypes.bfloat16
    x = np.asarray(x, np.float32)
    wqn = np.ascontiguousarray(np.asarray(Wq, np.float32).astype(bf16))
    wkn = np.ascontiguousarray(np.asarray(Wk, np.float32).astype(bf16))
    wvt = np.ascontiguousarray(np.asarray(Wv, np.float32).astype(bf16).T)
    bqc = np.ascontiguousarray(
        np.asarray(bq, np.float32).astype(bf16).reshape(KT, P).T)
    bvb = np.asarray(bv, np.float32).reshape(1, D).astype(bf16)
    # maskL[a, i] = 1 iff i in 32-block a; maskR[a, j] = NEG unless j in a
    blk = (np.arange(P) // 32)[None, :] == np.arange(4)[:, None]
    mskl = blk.astype(bf16)
    mskr = np.where(blk, 0.0, NEG).astype(bf16)
    in_maps = []
    for c in range(8):
        b, h = divmod(c, 2)
        xs = x[b, 4096 * h:4096 * h + 4096]
        xs = xs.reshape(64, 64, D)[:, ::2, :].reshape(2048, D).astype(bf16)
        # x.T in chunk-major rows: row 1024*c + d = x.T[d, 512c:512c+512]
        xt = np.ascontiguousarray(
            xs.T.reshape(D, TCH, FD).transpose(1, 0, 2)).reshape(TCH * D, FD)
        in_maps.append({"xt": xt, "wqn": wqn, "wkn": wkn, "wvt": wvt,
                        "bqc": bqc, "bvb": bvb, "mskl": mskl, "mskr": mskr})
    return in_maps


def kernel(x, Wq, bq, Wk, bk, Wv, bv):
    from concourse.bass_utils import run_bass_kernel_spmd

    nc = get_nc()
    in_maps = make_in_maps(x, Wq, bq, Wk, bk, Wv, bv)
    res = run_bass_kernel_spmd(nc, in_maps, core_ids=list(range(8)))
    _CACHE["last_res"] = res
    out = np.empty((4, 4096, D), np.float32)
    for c in range(8):
        b, h = divmod(c, 2)
        out[b, 2048 * h:2048 * h + 2048] = res.results[c]["out"]
    return out


# revision 21
# speedup vs baseline: 1.3486x; 1.0077x over previous
"""Dilated segment attention on 8 Trainium2 NeuronCores (Bass/Tile).

Problem: x:[4,8192,1024] fp32. Per 64-token segment, rows ::2 are kept
(32 tokens), projected with Wq/Wk/Wv (+bias), and full-dim attention is
computed within each segment. Output: [4,4096,1024] fp32.

Sharding: data-parallel. Core c handles batch c//2, sequence half c%2 ->
2048 dilated tokens = 64 segments. No collectives.

Key algebraic restructure: softmax over keys cancels every term of
q_j.k_i that is constant in the key index i, so

  softmax_i(q_j . k_i) = softmax_i( x_j A x_i^T + x_i . w ),
  A = Wq^T Wk,  w = bq Wk        (bk drops out entirely).

The kernel computes A on-device once per core (1024^3 MACs, started as
soon as the 4 MB of Wq/Wk bf16 lands) and replaces BOTH the q and k
projections with a single h-projection h = x A + w; simT[i,j] = x_i.h_j.
This cuts projection matmul work from 3 passes to 2 (plus the cheap A).

Host prep is layout/dtype only: weights passed bf16 (native Wq/Wk for
the A matmul, Wv^T for the v pass), x passed dilated+transposed+bf16 in
chunk-major [4*1024, 512] layout. All SBUF residents use single big
tiles so each load is ONE 1-2 MB DMA op (descriptor-efficient), halves
split across the two HWDGE rings (sync + scalar) in priority order
wk -> wq -> x chunk 0 / wv -> x chunks 1-3.

Per-core pipeline (all matmuls bf16, fp32 PSUM):
  - warm-up junk matmuls keep the PE HAM at 2.4 GHz until real work
  - w = bq Wk (64 tiny matmuls, needs only Wk), A = Wq^T Wk (128
    matmuls, N=512) -> A in SBUF bf16
  - per chunk c: v-pass (x stationary, Wv^T moving); h-pass (A
    stationary, x.T moving) with w fused into the ACT psum->sbuf
    epilogue; simT per 4-segment group as one packed 128x128 matmul
    over 8 k-tiles plus a rank-4 mask matmul that puts -30000 on the
    off-diagonal 32x32 blocks so one full-tile ACT exp yields the
    block-diagonal p (off-diag underflows to exactly 0); attn@v and the
    softmax denominator l (ones-column matmul) per token tile; final
    out = psum_av * (1/l) + bv in one DVE scalar_tensor_tensor, written
    out on the (by then idle) HWDGE rings.
"""

import numpy as np

P = 128
D = 1024
KT = 8    # d tiles of 128
OT = 8    # d_out tiles of 128
NTT = 16  # token tiles of 128 (2048 tokens per core)
FD = 512  # matmul moving free dim / psum bank
TCH = 4   # token chunks of 512
NEG = -30000.0  # off-diagonal mask; exp(scale*(sim+NEG)) underflows to 0

_CACHE = {}


def _build_nc():
    import os
    from contextlib import ExitStack

    import concourse.bass as bass
    import concourse.mybir as mybir
    import concourse.tile as tile
    from concourse import bacc

    KWARM = int(os.environ.get("KWARM", "60"))

    dt = mybir.dt
    AF = mybir.ActivationFunctionType
    ALU = mybir.AluOpType

    nc = bacc.Bacc("TRN2", target_bir_lowering=False, debug=False,
                   enable_asserts=False)

    # x.T, dilated, bf16, chunk-major: row 1024*c + d holds x.T[d, 512c:...]
    xt_d = nc.dram_tensor("xt", [TCH * D, FD], dt.bfloat16,
                          kind="ExternalInput")
    wq_d = nc.dram_tensor("wqn", [D, D], dt.bfloat16, kind="ExternalInput")
    wk_d = nc.dram_tensor("wkn", [D, D], dt.bfloat16, kind="ExternalInput")
    wv_d = nc.dram_tensor("wvt", [D, D], dt.bfloat16, kind="ExternalInput")
    bqc_d = nc.dram_tensor("bqc", [P, KT], dt.bfloat16, kind="ExternalInput")
    bvb_d = nc.dram_tensor("bvb", [1, D], dt.bfloat16, kind="ExternalInput")
    mskl_d = nc.dram_tensor("mskl", [4, P], dt.bfloat16, kind="ExternalInput")
    mskr_d = nc.dram_tensor("mskr", [4, P], dt.bfloat16, kind="ExternalInput")
    out_d = nc.dram_tensor("out", [2048, D], dt.float32, kind="ExternalOutput")

    scale = float(D) ** -0.5

    with tile.TileContext(nc) as tc, ExitStack() as ctx:
        consts = ctx.enter_context(tc.tile_pool(name="consts", bufs=1))
        resid = ctx.enter_context(tc.tile_pool(name="resid", bufs=1))
        outp = ctx.enter_context(tc.tile_pool(name="outp", bufs=3))
        rsbp = ctx.enter_context(tc.tile_pool(name="rsbp", bufs=2))

        ones_col = consts.tile([P, 1], dt.bfloat16, name="ones_col")
        ones_row = consts.tile([1, P], dt.bfloat16, name="ones_row")
        maskL = consts.tile([4, P], dt.bfloat16, name="maskL")
        maskR = consts.tile([4, P], dt.bfloat16, name="maskR")
        junk_w = consts.tile([P, P], dt.bfloat16, name="junk_w")
        junk_m = consts.tile([P, P], dt.bfloat16, name="junk_m")
        bqc_sb = consts.tile([P, KT], dt.bfloat16, name="bqc_sb")
        bvb_sb = consts.tile([1, D], dt.bfloat16, name="bvb_sb")
        w_sb = consts.tile([P, OT], dt.float32, name="w_sb")
        bv_rep = consts.tile([P, D], dt.float32, name="bv_rep")

        nc.vector.memset(ones_col[:], 1.0)
        nc.vector.memset(ones_row[:], 1.0)
        nc.vector.memset(junk_w[:], 0.0)
        nc.vector.memset(junk_m[:], 0.0)

        # big SBUF residents; each DMA below is one 1-2 MB op
        xTall = resid.tile([P, TCH * KT * FD], dt.bfloat16, name="xTall")
        wkall = resid.tile([P, KT * D], dt.bfloat16, name="wkall")
        wqall = resid.tile([P, KT * D], dt.bfloat16, name="wqall")
        wvall = resid.tile([P, KT * D], dt.bfloat16, name="wvall")
        A_sb = [resid.tile([P, D], dt.bfloat16, name=f"A{m}")
                for m in range(KT)]
        hT = [resid.tile([P, 2048], dt.bfloat16, name=f"hT{o}")
              for o in range(OT)]
        vv = [resid.tile([P, D], dt.bfloat16, name=f"v{t}") for t in range(NTT)]
        pT = [resid.tile([P, P], dt.bfloat16, name=f"pT{g}") for g in range(NTT)]

        def xT(k, lo, n):
            """columns [lo, lo+n) of d-stripe k of x.T (token index)."""
            c, j = divmod(lo, FD)
            assert j + n <= FD
            base = (KT * c + k) * FD + j
            return xTall[:, base:base + n]

        def wsl(wall, i, lo, n):
            return wall[:, D * i + lo:D * i + lo + n]

        # ---- DMA priority order. Tiny consts ride the idle SWDGE ring.
        # Wk/Wq go in 2-tile (256 KB) pieces, interleaved across the two
        # HWDGE rings in i order, so the i-outer A sweep starts as soon as
        # the first pair lands.  Then wv (scalar) and x chunks (sync).
        nc.gpsimd.dma_start(bqc_sb[:], bqc_d[:])
        nc.gpsimd.dma_start(bvb_sb[:], bvb_d[:])
        nc.gpsimd.dma_start(maskL[:], mskl_d[:])
        nc.gpsimd.dma_start(maskR[:], mskr_d[:])

        def load_w_tiles(eng, wall, w_d, lo, n):
            src = bass.AP(w_d, lo * P * D, [[D, P], [P * D, n], [1, D]])
            eng.dma_start(wall[:, lo * D:(lo + n) * D], src)

        # finest pieces first so the i-outer A sweep starts ASAP
        for i in range(2):
            load_w_tiles(nc.scalar, wkall, wk_d, i, 1)
            load_w_tiles(nc.sync, wqall, wq_d, i, 1)
        for pr in range(1, 4):
            load_w_tiles(nc.scalar, wkall, wk_d, 2 * pr, 2)
            load_w_tiles(nc.sync, wqall, wq_d, 2 * pr, 2)
        src = bass.AP(wv_d, 0, [[D, P], [P * D, KT], [1, D]])
        nc.scalar.dma_start(wvall[:], src)
        for c in range(TCH):
            src = bass.AP(xt_d, c * D * FD, [[FD, P], [P * FD, KT], [1, FD]])
            nc.sync.dma_start(xTall[:, KT * FD * c:KT * FD * (c + 1)], src)

        with tc.tile_pool(name="projp", bufs=3, space="PSUM") as projp, \
             tc.tile_pool(name="simp", bufs=2, space="PSUM") as simp, \
             tc.tile_pool(name="avp", bufs=2, space="PSUM") as avp, \
             tc.tile_pool(name="lp", bufs=1, space="PSUM") as lp:

            # ---- HAM warm-up: junk matmuls from t~0 so the PE clock is
            # at 2.4 GHz when the first real matmuls arrive.
            if KWARM:
                wps = projp.tile([P, FD], dt.float32, name="pps")
                for _ in range(KWARM):
                    nc.tensor.matmul(wps[:, 0:P], junk_w[:], junk_m[:],
                                     start=True, stop=True)

            # ---- A = Wq^T @ Wk  (A[a, b] = sum_o Wq[o, a] Wk[o, b]).
            # i-OUTER with 8 simultaneous accumulators (one psum bank each,
            # borrowed across all four pools = exactly 8 banks): the first
            # 4 i-steps contract the first halves of Wq/Wk, so the sweep
            # starts as soon as those 2 MB land instead of waiting for all
            # 4 MB.  w = bq @ Wk slots between the two half-sweeps, giving
            # the DVE time to evacuate sweep 0 before its banks are reused.
            def a_sweep(hf):
                acc = [projp.tile([P, FD], dt.float32, name="pps")
                       for _ in range(3)]
                acc += [simp.tile([P, FD], dt.float32, name="sps")
                        for _ in range(2)]
                acc += [avp.tile([P, FD], dt.float32, name="avs")
                        for _ in range(2)]
                acc += [lp.tile([P, FD], dt.float32, name="lps")]
                for i in range(KT):
                    for m in range(KT):
                        nc.tensor.matmul(acc[m][:],
                                         wsl(wqall, i, P * m, P),
                                         wsl(wkall, i, FD * hf, FD),
                                         start=(i == 0), stop=(i == KT - 1))
                for m in range(KT):
                    nc.vector.tensor_copy(A_sb[m][:, FD * hf:FD * hf + FD],
                                          acc[m][:])

            a_sweep(0)

            # ---- bv broadcast to all partitions via K=1 ones matmul
            # (needed only by the out epilogue; emitted here so the junk ->
            # sweep0 handoff isn't gated on the tiny bvb DMA)
            for dh in range(2):
                ps = projp.tile([P, FD], dt.float32, name="pps")
                nc.tensor.matmul(ps[:], ones_row[:],
                                 bvb_sb[:, FD * dh:FD * dh + FD],
                                 start=True, stop=True)
                nc.scalar.copy(bv_rep[:, FD * dh:FD * dh + FD], ps[:])

            # ---- w = bq @ Wk, column m at a time (psum [P, OT])
            w_ps = lp.tile([P, OT], dt.float32, name="lps")
            for m in range(OT):
                for i in range(KT):
                    nc.tensor.matmul(w_ps[:, m:m + 1],
                                     wsl(wkall, i, P * m, P),
                                     bqc_sb[:, i:i + 1],
                                     start=(i == 0), stop=(i == KT - 1))
            nc.vector.tensor_copy(w_sb[:], w_ps[:])

            a_sweep(1)

            def v_pass(t):
                pss = [projp.tile([P, FD], dt.float32, name="pps")
                       for _ in range(2)]
                for i in range(KT):
                    for dh in range(2):
                        nc.tensor.matmul(pss[dh][:],
                                         xT(i, P * t, P),
                                         wsl(wvall, i, FD * dh, FD),
                                         start=(i == 0), stop=(i == KT - 1))
                for dh in range(2):
                    nc.vector.tensor_copy(vv[t][:, FD * dh:FD * dh + FD],
                                          pss[dh][:])

            def av_group(t, eng):
                lps = lp.tile([P, 1], dt.float32, name="lps")
                nc.tensor.matmul(lps[:], pT[t][:], ones_col[:],
                                 start=True, stop=True)
                rsb = rsbp.tile([P, 1], dt.float32, name="rsb")
                nc.vector.reciprocal(rsb[:], lps[:])
                osb = outp.tile([P, D], dt.float32, name="osb")
                for dh in range(2):
                    avs = avp.tile([P, FD], dt.float32, name="avs")
                    nc.tensor.matmul(avs[:], pT[t][:],
                                     vv[t][:, FD * dh:FD * dh + FD],
                                     start=True, stop=True)
                    nc.vector.scalar_tensor_tensor(
                        osb[:, FD * dh:FD * dh + FD], avs[:], rsb[:],
                        bv_rep[:, FD * dh:FD * dh + FD],
                        ALU.mult, ALU.add)
                    eng.dma_start(
                        bass.AP(out_d, t * P * D + FD * dh,
                                [[D, P], [1, FD]]),
                        osb[:, FD * dh:FD * dh + FD])

            for c in range(TCH):
                # v for this chunk (needs only x chunk c + Wv).  The last
                # chunk's v tiles are instead interleaved with its attn@v
                # below, so the final STT/out epilogues hide behind v
                # matmuls instead of bunching DVE-bound at the very end.
                if c < TCH - 1:
                    for t in range(4 * c, 4 * c + 4):
                        v_pass(t)
                # h-pass for chunk c: h = x A + w, stored transposed
                for o in range(OT):
                    psH = projp.tile([P, FD], dt.float32, name="pps")
                    for i in range(KT):
                        nc.tensor.matmul(psH[:],
                                         A_sb[i][:, P * o:P * o + P],
                                         xT(i, FD * c, FD),
                                         start=(i == 0), stop=(i == KT - 1))
                    nc.scalar.activation(hT[o][:, FD * c:FD * c + FD],
                                         psH[:], AF.Identity,
                                         bias=w_sb[:, o:o + 1], scale=1.0)
                # simT + exp for this chunk's 4 groups
                for g in range(4 * c, 4 * c + 4):
                    sps = simp.tile([P, P], dt.float32, name="sps")
                    nc.tensor.matmul(sps[:], maskL[:], maskR[:],
                                     start=True, stop=False)
                    for kk in range(KT):
                        nc.tensor.matmul(sps[:],
                                         xT(kk, P * g, P),
                                         hT[kk][:, P * g:P * g + P],
                                         start=False, stop=(kk == KT - 1))
                    nc.scalar.activation(pT[g][:], sps[:], AF.Exp,
                                         bias=0.0, scale=scale)
                # attn@v for this chunk; out rides the idle HWDGE rings
                for t in range(4 * c, 4 * c + 4):
                    if c == TCH - 1:
                        v_pass(t)
                    av_group(t, nc.sync if t % 2 == 0 else nc.scalar)

    nc.compile()
    return nc


def get_nc():
    if "nc" not in _CACHE:
        _CACHE["nc"] = _build_nc()
    return _CACHE["nc"]


def make_in_maps(x, Wq, bq, Wk, bk, Wv, bv):
    import ml_dtypes

    bf16 = ml_dtypes.bfloat16
    x = np.asarray(x, np.float32)
    wqn = np.ascontiguousarray(np.asarray(Wq, np.float32).astype(bf16))
    wkn = np.ascontiguousarray(np.asarray(Wk, np.float32).astype(bf16))
    wvt = np.ascontiguousarray(np.asarray(Wv, np.float32).astype(bf16).T)
    bqc = np.ascontiguousarray(
        np.asarray(bq, np.float32).astype(bf16).reshape(KT, P).T)
    bvb = np.asarray(bv, np.float32).reshape(1, D).astype(bf16)
    # maskL[a, i] = 1 iff i in 32-block a; maskR[a, j] = NEG unless j in a
    blk = (np.arange(P) // 32)[None, :] == np.arange(4)[:, None]
    mskl = blk.astype(bf16)
    mskr = np.where(blk, 0.0, NEG).astype(bf16)
    in_maps = []
    for c in range(8):
        b, h = divmod(c, 2)
        xs = x[b, 4096 * h:4096 * h + 4096]
        xs = xs.reshape(64, 64, D)[:, ::2, :].reshape(2048, D).astype(bf16)
        # x.T in chunk-major rows: row 1024*c + d = x.T[d, 512c:512c+512]
        xt = np.ascontiguousarray(
            xs.T.reshape(D, TCH, FD).transpose(1, 0, 2)).reshape(TCH * D, FD)
        in_maps.append({"xt": xt, "wqn": wqn, "wkn": wkn, "wvt": wvt,
                        "bqc": bqc, "bvb": bvb, "mskl": mskl, "mskr": mskr})
    return in_maps


def kernel(x, Wq, bq, Wk, bk, Wv, bv):
    from concourse.bass_utils import run_bass_kernel_spmd

    nc = get_nc()
    in_maps = make_in_maps(x, Wq, bq, Wk, bk, Wv, bv)
    try:
        res = run_bass_kernel_spmd(nc, in_maps, core_ids=list(range(8)))
    except Exception:
        # one retry against a transiently wedged device
        res = run_bass_kernel_spmd(nc, in_maps, core_ids=list(range(8)))
    _CACHE["last_res"] = res
    out = np.empty((4, 4096, D), np.float32)
    for c in range(8):
        b, h = divmod(c, 2)
        out[b, 2048 * h:2048 * h + 2048] = res.results[c]["out"]
    return out
